# revision 1
# baseline (speedup 1.0000x reference)
"""GQA attention kernel for 8 TRN2 NeuronCores.

Problem: x[4,2048,1024], 16 Q heads / 4 KV heads, head_dim 64 (torch-Linear
style projections, softmax(QK^T/8)V, output projection + bias).

Sharding: core c handles (batch b = c//2, half h2 = c%2) where a half is
2 KV heads = 8 Q heads = 512 hidden dims. Every core computes a partial
output projection over its 512 hidden dims; pairs (2b, 2b+1) AllReduce-add
their partials on-device, host reads the even core's buffer.

Per-core layouts (prepared on host, bf16):
  xt  [1024, 2048]  x[b]^T              (embed dim on partitions)
  wqt [1024, 512]   wq[512h2:512h2+512]^T
  wkt [1024, 128]   wk[128h2:128h2+128]^T
  wvt [1024, 128]   wv rows likewise
  wot [512, 1024]   wo^T rows for this half's hidden dims
  bo2 [1, 1024]     0.5 * bo (each pair member adds half -> sum = bo)

Inside: q^T/k^T computed in [dim, token] layout so QK^T needs no
transposes; S^T tiles [keys=128, queries=512] are exp'd on ACT straight
from PSUM; V is augmented with a ones column so the AV matmul also
produces the softmax denominators; normalization is deferred to the
attention output (gpsimd partition_broadcast of the reciprocal row).
"""

import sys
import numpy as np
from contextlib import ExitStack

sys.path.insert(0, "/opt/trn_rl_repo")

import ml_dtypes

from concourse import bass, tile, mybir


# ---------------------------------------------------------------------------
# This walrus build encodes at most 1-2 sync waits per instruction; the stock
# TileContext tail drain packs one wait per live proc onto a single Drain and
# fails codegen ("Too many sync wait commands"). Spread the waits over SP nop
# carriers instead.
def _patched_drain_and_barrier(self, tick_clock, wait_clock):
    from concourse.vector_clock import ScopedClock, VectorClock

    nc = self.nc
    gc = tick_clock.global_clock
    n = len(gc)
    for proc in range(n):
        t = gc[proc]
        if t <= 0:
            continue
        carrier = nc.sync.nop(nofuse=True)
        req = VectorClock([t if i == proc else 0 for i in range(n)])
        wait_clock.add_sem_waits(carrier.ins, ScopedClock({None: req}))
    nc.sync.drain()
    nc.all_engine_barrier()
    assert self.sems is not None
    popped = nc._tile_sem_poison_stack.pop()
    assert popped is self._sem_poison
    nc.clear_and_free_semaphores(list(self.sems.allocated().values()))
    nc.all_engine_barrier()


tile.TileContext._drain_and_barrier = _patched_drain_and_barrier


def _split_excess_waits(nc, max_waits=1):
    """Hoist all but one sync wait per instruction onto dedicated
    EventSemaphore carriers placed immediately before it on the same engine
    (same blocking semantics, one wait per encoded instruction)."""
    n_new = 0
    for bb in nc.main_func.blocks:
        il = list(bb.instructions)
        out = []
        changed = False
        for ins in il:
            si = ins.sync_info
            if si is not None:
                w = list(si.on_wait)
                if len(w) > max_waits:
                    for extra in w[max_waits:]:
                        ev = mybir.InstEventSemaphore(
                            name=f"{ins.name}-wsp{n_new}", engine=ins.engine)
                        n_new += 1
                        ev.sync_info = type(si)(on_wait=[extra], on_update=[])
                        nc.register_instruction(ev, overwrite=True)
                        out.append(ev)
                    si.on_wait = w[:max_waits]
                    changed = True
            out.append(ins)
        if changed:
            bb.instructions = out
# ---------------------------------------------------------------------------

B, N, D = 4, 2048, 1024
DH = 64  # head dim
HID = 512  # hidden dims per core (8 q heads)
NCORES = 8
P = 128
SCALE = DH ** -0.5
BF16 = mybir.dt.bfloat16
F32 = mybir.dt.float32

NB = N // P  # 16 token blocks of 128
NK = D // P  # 8 contraction chunks of 128
NQB = 4  # n blocks of 512 for attention moving dim
VW = 256  # v chunk width: [64 v_h0 | 64 ones | 64 v_h1 | 64 ones]


def build_nc(st_bufs=3, pt_bufs=4, trace_friendly=False):
    nc = bass.Bass(target_bir_lowering=False, debug=False, num_devices=NCORES)

    xt = nc.declare_dram_parameter("xt", [D, N], BF16, isOutput=False)
    wqt = nc.declare_dram_parameter("wqt", [D, HID], BF16, isOutput=False)
    wkt = nc.declare_dram_parameter("wkt", [D, P], BF16, isOutput=False)
    wvt = nc.declare_dram_parameter("wvt", [D, P], BF16, isOutput=False)
    wot = nc.declare_dram_parameter("wot", [HID, D], BF16, isOutput=False)
    bo2 = nc.declare_dram_parameter("bo2", [1, D], F32, isOutput=False)
    out_p = nc.declare_dram_parameter("out_p", [N, D], F32, isOutput=True)

    ob = nc.dram_tensor("ob", [N, D], F32)  # partial o-proj (collective in)
    rb = nc.dram_tensor("rb", [N, D], F32)  # pair-summed (collective out)

    with tile.TileContext(nc) as tc, ExitStack() as ctx:
        const = ctx.enter_context(tc.tile_pool(name="const", bufs=1))
        work = ctx.enter_context(tc.tile_pool(name="work", bufs=1))
        ppool = ctx.enter_context(tc.tile_pool(name="ppool", bufs=3, space="PSUM"))
        stpool = ctx.enter_context(tc.tile_pool(name="stp", bufs=st_bufs, space="PSUM"))
        avpool = ctx.enter_context(tc.tile_pool(name="avp", bufs=2, space="PSUM"))
        ptpool = ctx.enter_context(tc.tile_pool(name="ptp", bufs=pt_bufs))
        smallp = ctx.enter_context(tc.tile_pool(name="smallp", bufs=3))
        outp = ctx.enter_context(tc.tile_pool(name="outp", bufs=3))

        # ---- load inputs -------------------------------------------------
        xt_sb = const.tile([P, NK * N], BF16)
        for kc in range(NK):
            nc.sync.dma_start(out=xt_sb[:, kc * N:(kc + 1) * N],
                              in_=xt[kc * P:(kc + 1) * P, :])
        wqt_sb = const.tile([P, NK * HID], BF16)
        wkt_sb = const.tile([P, NK * P], BF16)
        wvt_sb = const.tile([P, NK * P], BF16)
        wot_sb = const.tile([P, 4 * D], BF16)
        for kc in range(NK):
            nc.sync.dma_start(out=wqt_sb[:, kc * HID:(kc + 1) * HID],
                              in_=wqt[kc * P:(kc + 1) * P, :])
            nc.sync.dma_start(out=wkt_sb[:, kc * P:(kc + 1) * P],
                              in_=wkt[kc * P:(kc + 1) * P, :])
            nc.sync.dma_start(out=wvt_sb[:, kc * P:(kc + 1) * P],
                              in_=wvt[kc * P:(kc + 1) * P, :])
        for ic in range(4):
            nc.sync.dma_start(out=wot_sb[:, ic * D:(ic + 1) * D],
                              in_=wot[ic * P:(ic + 1) * P, :])
        bo_row = const.tile([1, D], F32)
        nc.sync.dma_start(out=bo_row[:], in_=bo2[0:1, :])
        ones_row = const.tile([1, P], F32)
        nc.vector.memset(ones_row[:], 1.0)
        # partition-broadcast via PE outer product (gpsimd InstISA is not
        # supported by this walrus build)
        bo_bc = const.tile([P, D], F32)
        for jh in range(2):
            bps = ppool.tile([P, 512], F32, tag="proj")
            nc.tensor.matmul(bps[:], lhsT=ones_row[:, 0:P],
                             rhs=bo_row[:, jh * 512:(jh + 1) * 512],
                             start=True, stop=True)
            nc.vector.tensor_copy(bo_bc[:, jh * 512:(jh + 1) * 512], bps[:])

        # ---- projections -------------------------------------------------
        # q^T [512, 2048] as 4 partition-blocks; k^T [128, 2048]; v natural.
        qt_sb = work.tile([P, 4 * N], BF16, tag="qt")
        kt_sb = work.tile([P, N], BF16, tag="kt")
        v_sb = work.tile([P, NB * VW], BF16, tag="v")
        nc.vector.memset(v_sb[:], 1.0)  # ones columns survive the copies

        for mb in range(4):  # q output-dim blocks
            for nb in range(NQB):
                ps = ppool.tile([P, 512], F32, tag="proj")
                for kc in range(NK):
                    nc.tensor.matmul(
                        ps[:],
                        lhsT=wqt_sb[:, kc * HID + mb * P: kc * HID + (mb + 1) * P],
                        rhs=xt_sb[:, kc * N + nb * 512: kc * N + (nb + 1) * 512],
                        start=(kc == 0), stop=(kc == NK - 1),
                    )
                nc.vector.tensor_copy(
                    qt_sb[:, mb * N + nb * 512: mb * N + (nb + 1) * 512], ps[:])

        for nb in range(NQB):
            ps = ppool.tile([P, 512], F32, tag="proj")
            for kc in range(NK):
                nc.tensor.matmul(
                    ps[:],
                    lhsT=wkt_sb[:, kc * P:(kc + 1) * P],
                    rhs=xt_sb[:, kc * N + nb * 512: kc * N + (nb + 1) * 512],
                    start=(kc == 0), stop=(kc == NK - 1),
                )
            nc.vector.tensor_copy(kt_sb[:, nb * 512:(nb + 1) * 512], ps[:])

        for mb in range(NB):  # v in natural [token, dim] layout
            ps = ppool.tile([P, P], F32, tag="proj")
            for kc in range(NK):
                nc.tensor.matmul(
                    ps[:],
                    lhsT=xt_sb[:, kc * N + mb * P: kc * N + (mb + 1) * P],
                    rhs=wvt_sb[:, kc * P:(kc + 1) * P],
                    start=(kc == 0), stop=(kc == NK - 1),
                )
            nc.vector.tensor_copy(v_sb[:, mb * VW: mb * VW + 64], ps[:, 0:64])
            nc.vector.tensor_copy(v_sb[:, mb * VW + 128: mb * VW + 192], ps[:, 64:128])

        # ---- attention ---------------------------------------------------
        # hidden^T [512, 2048] bf16, normalized attention outputs
        hid_sb = work.tile([P, 4 * N], BF16, tag="hid")

        # head h lives in q/hid block h%4 at partition half h//4 == its kv
        # head's half in kt (wqt cols / wot rows are host-reordered to match),
        # so the QK matmul's lhsT and rhs share a base partition.
        for h in range(8):  # local q heads
            kv = h // 4  # local kv head
            qp = 64 * kv  # partition offset in qt block
            qb = h % 4  # qt partition-block
            hp = qp
            hb = qb
            for nb in range(NQB):
                # av rows 0:64 = unnormalized attention out (v columns);
                # rows 64:128 = softmax denominators, broadcast across 64
                # partitions for free by the ones columns of v_aug.
                av = avpool.tile([P, 512], F32, tag="av")
                for mc in range(NB):
                    st = stpool.tile([P, 512], F32, tag="st")
                    nc.tensor.matmul(
                        st[:],
                        lhsT=kt_sb[64 * kv:64 * kv + 64, mc * P:(mc + 1) * P],
                        rhs=qt_sb[qp:qp + 64, qb * N + nb * 512: qb * N + (nb + 1) * 512],
                        start=True, stop=True,
                    )
                    pt = ptpool.tile([P, 512], BF16, tag="pt")
                    nc.scalar.activation(pt[:], st[:],
                                         mybir.ActivationFunctionType.Exp,
                                         scale=SCALE)
                    nc.tensor.matmul(
                        av[:],
                        lhsT=v_sb[:, mc * VW + 128 * kv: mc * VW + 128 * kv + 128],
                        rhs=pt[:],
                        start=(mc == 0), stop=(mc == NB - 1),
                    )
                den = smallp.tile([64, 512], F32, tag="den")
                nc.vector.reciprocal(den[:], av[64:128, :])
                nc.vector.tensor_tensor(
                    out=hid_sb[hp:hp + 64, hb * N + nb * 512: hb * N + (nb + 1) * 512],
                    in0=av[0:64, :], in1=den[:],
                    op=mybir.AluOpType.mult,
                )

        # ---- output projection ------------------------------------------
        for tb in range(NB):
            ot = outp.tile([P, D], F32, tag="osb")
            for jh in range(2):
                ps = ppool.tile([P, 512], F32, tag="proj")
                for ic in range(4):
                    nc.tensor.matmul(
                        ps[:],
                        lhsT=hid_sb[:, ic * N + tb * P: ic * N + (tb + 1) * P],
                        rhs=wot_sb[:, ic * D + jh * 512: ic * D + (jh + 1) * 512],
                        start=(ic == 0), stop=(ic == 3),
                    )
                nc.vector.tensor_tensor(
                    out=ot[:, jh * 512:(jh + 1) * 512],
                    in0=ps[:], in1=bo_bc[:, jh * 512:(jh + 1) * 512],
                    op=mybir.AluOpType.add,
                )
            nc.sync.dma_start(out=ob[tb * P:(tb + 1) * P, :], in_=ot[:])

        # ---- pair all-reduce + output -----------------------------------
        nc.gpsimd.collective_compute(
            "AllReduce", mybir.AluOpType.add,
            replica_groups=[[0, 1], [2, 3], [4, 5], [6, 7]],
            ins=[ob.ap().opt()], outs=[rb.ap().opt()],
        )
        for tb in range(4):
            nc.sync.dma_start(out=out_p[tb * 512:(tb + 1) * 512, :],
                              in_=rb[tb * 512:(tb + 1) * 512, :])

    _split_excess_waits(nc)
    return nc


def make_in_maps(x, wq, wk, wv, wo, bo):
    bf = ml_dtypes.bfloat16
    # local head h -> device slot (block h%4, half h//4): permuted head order
    hperm = [0, 4, 1, 5, 2, 6, 3, 7]
    dperm = np.concatenate([np.arange(64 * h, 64 * h + 64) for h in hperm])
    in_maps = []
    for c in range(NCORES):
        b, h2 = c // 2, c % 2
        wq_c = wq[HID * h2:HID * (h2 + 1)][dperm]  # [512, 1024] permuted rows
        wot_c = wo.T[HID * h2:HID * (h2 + 1)][dperm]  # [512, 1024] same perm
        in_maps.append({
            "xt": np.ascontiguousarray(x[b].T).astype(bf),
            "wqt": np.ascontiguousarray(wq_c.T).astype(bf),
            "wkt": np.ascontiguousarray(wk[P * h2:P * (h2 + 1)].T).astype(bf),
            "wvt": np.ascontiguousarray(wv[P * h2:P * (h2 + 1)].T).astype(bf),
            "wot": np.ascontiguousarray(wot_c).astype(bf),
            "bo2": (0.5 * bo).astype(np.float32).reshape(1, D),
        })
    return in_maps


_CACHED_NC = None


def kernel(x, wq, wk, wv, wo, bo, _trace=False, _trace_kwargs=None):
    global _CACHED_NC
    from concourse.bass_utils import run_bass_kernel_spmd

    if _CACHED_NC is None:
        _CACHED_NC = build_nc()
    nc = _CACHED_NC

    in_maps = make_in_maps(
        np.asarray(x, np.float32), np.asarray(wq, np.float32),
        np.asarray(wk, np.float32), np.asarray(wv, np.float32),
        np.asarray(wo, np.float32), np.asarray(bo, np.float32))

    res = run_bass_kernel_spmd(
        nc, in_maps, core_ids=list(range(NCORES)),
        trace=_trace, **(_trace_kwargs or {}))

    out = np.empty((B, N, D), np.float32)
    for b in range(B):
        out[b] = res.results[2 * b]["out_p"]
    if _trace:
        kernel._last_results = res
    return out



# revision 5
# speedup vs baseline: 1.4092x; 1.4092x over previous
"""GQA attention kernel for 8 TRN2 NeuronCores (v2).

Problem: x[4,2048,1024], 16 Q heads / 4 KV heads, head_dim 64 (torch-Linear
style projections, softmax(QK^T/8)V, output projection + bias).

Sharding: core c handles (batch b = c//2, half h2 = c%2) where a half is
2 KV heads = 8 Q heads = 512 hidden dims. Per 512-token chunk, each core
computes the partial output projection over its 512 hidden dims (bf16,
pre-bias); the pair ReduceScatters it so each member ends up with the final
sum for a disjoint 256-token quarter, adds the bias, and writes those rows
to out_p. The program is identical on all cores (which quarter a core gets
falls out of its replica-group rank); the host stitches by core parity.

Attention inner loop (per query chunk nb, per head-pair (kv0 head, kv1
head)): the two K=64 QK matmuls sit at SBUF partition bases 0/64, so they
auto-derive tile_position (0,0)/(64,0) and run concurrently in disjoint PE
row groups; their [128,512] score tiles land in adjacent PSUM banks and are
exp'd by ONE FD=1024 ACTIVATE (ACT is the bottleneck engine; its ~300-cycle
fixed overhead is amortized). V is augmented with ones columns so the AV
matmul also produces softmax denominators broadcast across 64 partitions
for free. av tiles are copied to SBUF immediately (frees PSUM early) and
the expensive DVE reciprocals (8 cyc/elem) are deferred and woven between
later pairs so they never stall the QK->exp->AV stream.
"""

import sys
import numpy as np
from contextlib import ExitStack

sys.path.insert(0, "/opt/trn_rl_repo")

import ml_dtypes

from concourse import bass, tile, mybir


# ---------------------------------------------------------------------------
# This walrus build encodes at most 1-2 sync waits per instruction; the stock
# TileContext tail drain packs one wait per live proc onto a single Drain and
# fails codegen ("Too many sync wait commands"). Spread the waits over SP nop
# carriers instead.
def _patched_drain_and_barrier(self, tick_clock, wait_clock):
    from concourse.vector_clock import ScopedClock, VectorClock

    nc = self.nc
    gc = tick_clock.global_clock
    n = len(gc)
    for proc in range(n):
        t = gc[proc]
        if t <= 0:
            continue
        carrier = nc.sync.nop(nofuse=True)
        req = VectorClock([t if i == proc else 0 for i in range(n)])
        wait_clock.add_sem_waits(carrier.ins, ScopedClock({None: req}))
    nc.sync.drain()
    nc.all_engine_barrier()
    assert self.sems is not None
    popped = nc._tile_sem_poison_stack.pop()
    assert popped is self._sem_poison
    nc.clear_and_free_semaphores(list(self.sems.allocated().values()))
    nc.all_engine_barrier()


tile.TileContext._drain_and_barrier = _patched_drain_and_barrier


def _split_excess_waits(nc, max_waits=1):
    """Hoist all but one sync wait per instruction onto dedicated
    EventSemaphore carriers placed immediately before it on the same engine
    (same blocking semantics, one wait per encoded instruction)."""
    n_new = 0
    for bb in nc.main_func.blocks:
        il = list(bb.instructions)
        out = []
        changed = False
        for ins in il:
            si = ins.sync_info
            if si is not None:
                w = list(si.on_wait)
                if len(w) > max_waits:
                    for extra in w[max_waits:]:
                        ev = mybir.InstEventSemaphore(
                            name=f"{ins.name}-wsp{n_new}", engine=ins.engine)
                        n_new += 1
                        ev.sync_info = type(si)(on_wait=[extra], on_update=[])
                        nc.register_instruction(ev, overwrite=True)
                        out.append(ev)
                    si.on_wait = w[:max_waits]
                    changed = True
            out.append(ins)
        if changed:
            bb.instructions = out
# ---------------------------------------------------------------------------

B, N, D = 4, 2048, 1024
DH = 64  # head dim
HID = 512  # hidden dims per core (8 q heads)
NCORES = 8
P = 128
SCALE = DH ** -0.5
BF16 = mybir.dt.bfloat16
F32 = mybir.dt.float32

NB = N // P  # 16 key blocks of 128
NK = D // P  # 8 contraction chunks of 128
NQB = 4  # n query chunks of 512
VW = 256  # v chunk width: [64 v_kv0 | 64 ones | 64 v_kv1 | 64 ones]
OWN = 256  # tokens of each 512-chunk this core ends up with after RS

RG = [[0, 1], [2, 3], [4, 5], [6, 7]]


def build_nc():
    nc = bass.Bass(target_bir_lowering=False, debug=False, num_devices=NCORES)

    xt = nc.declare_dram_parameter("xt", [D, N], BF16, isOutput=False)
    wqt = nc.declare_dram_parameter("wqt", [D, HID], BF16, isOutput=False)
    wkt = nc.declare_dram_parameter("wkt", [D, P], BF16, isOutput=False)
    wvt = nc.declare_dram_parameter("wvt", [D, P], BF16, isOutput=False)
    wot = nc.declare_dram_parameter("wot", [HID, D], BF16, isOutput=False)
    bo_in = nc.declare_dram_parameter("bo_in", [1, D], F32, isOutput=False)
    out_p = nc.declare_dram_parameter("out_p", [N, D], F32, isOutput=True)

    # per-chunk partial o-proj (bf16, pre-bias) and its pair ReduceScatter
    po = [nc.dram_tensor(f"po{k}", [512, D], BF16) for k in range(NQB)]
    rs = [nc.dram_tensor(f"rs{k}", [OWN, D], BF16) for k in range(NQB)]

    with tile.TileContext(nc) as tc, ExitStack() as ctx:
        const = ctx.enter_context(tc.tile_pool(name="const", bufs=1))
        work = ctx.enter_context(tc.tile_pool(name="work", bufs=1))
        # PSUM: st 2x[128,1024] (4 banks) + av 3 + proj 1 = 8 banks
        stpool = ctx.enter_context(tc.tile_pool(name="stp", bufs=2, space="PSUM"))
        avpool = ctx.enter_context(tc.tile_pool(name="avp", bufs=3, space="PSUM"))
        ppool = ctx.enter_context(tc.tile_pool(name="ppool", bufs=1, space="PSUM"))
        ptpool = ctx.enter_context(tc.tile_pool(name="ptp", bufs=3))
        avsb = ctx.enter_context(tc.tile_pool(name="avsb", bufs=8))
        rpool = ctx.enter_context(tc.tile_pool(name="rp", bufs=3))
        popool = ctx.enter_context(tc.tile_pool(name="pop", bufs=3))
        finp = ctx.enter_context(tc.tile_pool(name="finp", bufs=2))

        # ---- load inputs -------------------------------------------------
        xt_sb = const.tile([P, NK * N], BF16)
        wkt_sb = const.tile([P, NK * P], BF16)
        for kc in range(NK):
            nc.sync.dma_start(out=wkt_sb[:, kc * P:(kc + 1) * P],
                              in_=wkt[kc * P:(kc + 1) * P, :])
            nc.sync.dma_start(out=xt_sb[:, kc * N:(kc + 1) * N],
                              in_=xt[kc * P:(kc + 1) * P, :])
        wqt_sb = const.tile([P, NK * HID], BF16)
        wvt_sb = const.tile([P, NK * P], BF16)
        for kc in range(NK):
            nc.sync.dma_start(out=wqt_sb[:, kc * HID:(kc + 1) * HID],
                              in_=wqt[kc * P:(kc + 1) * P, :])
            nc.sync.dma_start(out=wvt_sb[:, kc * P:(kc + 1) * P],
                              in_=wvt[kc * P:(kc + 1) * P, :])
        wot_sb = const.tile([P, 4 * D], BF16)
        for ic in range(4):
            nc.sync.dma_start(out=wot_sb[:, ic * D:(ic + 1) * D],
                              in_=wot[ic * P:(ic + 1) * P, :])
        bo_row = const.tile([1, D], F32)
        nc.sync.dma_start(out=bo_row[:], in_=bo_in[0:1, :])
        ones_row = const.tile([1, P], F32)
        nc.vector.memset(ones_row[:], 1.0)
        # partition-broadcast of the bias row via PE outer product
        bo_bc = const.tile([P, D], F32)
        for jh in range(2):
            bps = ppool.tile([P, 512], F32, tag="proj")
            nc.tensor.matmul(bps[:], lhsT=ones_row[:, 0:P],
                             rhs=bo_row[:, jh * 512:(jh + 1) * 512],
                             start=True, stop=True)
            nc.vector.tensor_copy(bo_bc[:, jh * 512:(jh + 1) * 512], bps[:])

        # ---- projections -------------------------------------------------
        # q^T [512, 2048] as 4 partition-blocks; k^T [128, 2048]; v natural.
        qt_sb = work.tile([P, 4 * N], BF16, tag="qt")
        kt_sb = work.tile([P, N], BF16, tag="kt")
        v_sb = work.tile([P, NB * VW], BF16, tag="v")
        nc.vector.memset(v_sb[:], 1.0)  # ones columns survive the copies

        def k_chain(nb):
            ps = avpool.tile([P, 512], F32, tag="av")
            for kc in range(NK):
                nc.tensor.matmul(
                    ps[:],
                    lhsT=wkt_sb[:, kc * P:(kc + 1) * P],
                    rhs=xt_sb[:, kc * N + nb * 512: kc * N + (nb + 1) * 512],
                    start=(kc == 0), stop=(kc == NK - 1),
                )
            nc.vector.tensor_copy(kt_sb[:, nb * 512:(nb + 1) * 512], ps[:])

        def v_chain(mb):
            ps = avpool.tile([P, 512], F32, tag="av")
            for kc in range(NK):
                nc.tensor.matmul(
                    ps[:, 0:P],
                    lhsT=xt_sb[:, kc * N + mb * P: kc * N + (mb + 1) * P],
                    rhs=wvt_sb[:, kc * P:(kc + 1) * P],
                    start=(kc == 0), stop=(kc == NK - 1),
                )
            nc.vector.tensor_copy(v_sb[:, mb * VW: mb * VW + 64], ps[:, 0:64])
            nc.vector.tensor_copy(v_sb[:, mb * VW + 128: mb * VW + 192],
                                  ps[:, 64:128])

        def q_chain(mb, nb):
            ps = ppool.tile([P, 512], F32, tag="proj")
            for kc in range(NK):
                nc.tensor.matmul(
                    ps[:],
                    lhsT=wqt_sb[:, kc * HID + mb * P: kc * HID + (mb + 1) * P],
                    rhs=xt_sb[:, kc * N + nb * 512: kc * N + (nb + 1) * 512],
                    start=(kc == 0), stop=(kc == NK - 1),
                )
            nc.vector.tensor_copy(
                qt_sb[:, mb * N + nb * 512: mb * N + (nb + 1) * 512], ps[:])

        for nb in range(NQB):
            k_chain(nb)
        for mb in range(NB):
            v_chain(mb)
        for mb in range(4):
            q_chain(mb, 0)

        # ---- attention ---------------------------------------------------
        # hidden^T [512, 2048] bf16, normalized attention outputs.
        # head h lives in q/hid block h%4 at partition half 64*(h//4), which
        # equals its kv head's half in kt (host-reordered weights).
        hid_sb = work.tile([P, 4 * N], BF16, tag="hid")

        avs_tiles = {}  # (h, nb) -> SBUF av tile [128,512] f32

        def attn_pair(p, nb):
            """Heads hA=p (kv0, partitions 0:64) and hB=p+4 (kv1, 64:128)."""
            avA = avpool.tile([P, 512], F32, tag="av")
            avB = avpool.tile([P, 512], F32, tag="av")
            for mc in range(NB):
                st = stpool.tile([P, 1024], F32, tag="st")
                nc.tensor.matmul(
                    st[:, 0:512],
                    lhsT=kt_sb[0:64, mc * P:(mc + 1) * P],
                    rhs=qt_sb[0:64, p * N + nb * 512: p * N + (nb + 1) * 512],
                    start=True, stop=True,
                )
                nc.tensor.matmul(
                    st[:, 512:1024],
                    lhsT=kt_sb[64:128, mc * P:(mc + 1) * P],
                    rhs=qt_sb[64:128, p * N + nb * 512: p * N + (nb + 1) * 512],
                    start=True, stop=True,
                )
                pt = ptpool.tile([P, 1024], BF16, tag="pt")
                nc.scalar.activation(pt[:], st[:],
                                     mybir.ActivationFunctionType.Exp,
                                     scale=SCALE)
                nc.tensor.matmul(
                    avA[:],
                    lhsT=v_sb[:, mc * VW: mc * VW + P],
                    rhs=pt[:, 0:512],
                    start=(mc == 0), stop=(mc == NB - 1),
                )
                nc.tensor.matmul(
                    avB[:],
                    lhsT=v_sb[:, mc * VW + P: mc * VW + 2 * P],
                    rhs=pt[:, 512:1024],
                    start=(mc == 0), stop=(mc == NB - 1),
                )
            sA = avsb.tile([P, 512], F32, tag="avs")
            sB = avsb.tile([P, 512], F32, tag="avs")
            nc.vector.tensor_copy(sA[:], avA[:])
            nc.vector.tensor_copy(sB[:], avB[:])
            avs_tiles[(p, nb)] = sA
            avs_tiles[(p + 4, nb)] = sB

        def normalize(h, nb):
            """hid[h-half, block h%4, nb chunk] = av_out * 1/den."""
            s = avs_tiles.pop((h, nb))
            hp = 64 * (h // 4)
            hb = h % 4
            rt = rpool.tile([64, 512], F32, tag="rt")
            nc.vector.reciprocal(rt[:], s[64:128, :])
            nc.vector.tensor_tensor(
                out=hid_sb[hp:hp + 64,
                           hb * N + nb * 512: hb * N + (nb + 1) * 512],
                in0=s[0:64, :], in1=rt[:],
                op=mybir.AluOpType.mult,
            )

        def oproj_chain(nb, tb, jh, psum_pool):
            """Partial o-proj (local 512 hid dims, pre-bias) for tokens
            [nb*512 + tb*128, +128), out dims [jh*512, +512), cast bf16."""
            ps = psum_pool.tile([P, 512], F32, tag=("proj" if psum_pool is ppool else "av"))
            for ic in range(4):
                nc.tensor.matmul(
                    ps[:],
                    lhsT=hid_sb[:, ic * N + nb * 512 + tb * P:
                                ic * N + nb * 512 + (tb + 1) * P],
                    rhs=wot_sb[:, ic * D + jh * 512: ic * D + (jh + 1) * 512],
                    start=(ic == 0), stop=(ic == 3),
                )
            pob = popool.tile([P, 512], BF16, tag="po")
            nc.vector.tensor_copy(pob[:], ps[:])
            nc.sync.dma_start(
                out=po[nb][tb * P:(tb + 1) * P, jh * 512:(jh + 1) * 512],
                in_=pob[:])

        def rs_issue(nb):
            nc.gpsimd.collective_compute(
                "ReduceScatter", mybir.AluOpType.add,
                replica_groups=RG,
                ins=[po[nb].ap().opt()], outs=[rs[nb].ap().opt()],
            )

        def rs_finish(nb):
            """Read back our 256-token quarter, add bias, write out_p."""
            for tb in range(2):
                rsb = finp.tile([P, D], BF16, tag="rsb")
                nc.sync.dma_start(out=rsb[:], in_=rs[nb][tb * P:(tb + 1) * P, :])
                ot = finp.tile([P, D], F32, tag="ot")
                nc.vector.tensor_tensor(out=ot[:], in0=rsb[:], in1=bo_bc[:],
                                        op=mybir.AluOpType.add)
                r0 = nb * 512 + tb * P
                nc.sync.dma_start(out=out_p[r0:r0 + P, :], in_=ot[:])

        for nb in range(NQB):
            for p in range(4):
                attn_pair(p, nb)
                if p == 0:
                    if nb > 0:
                        normalize(3, nb - 1)
                        normalize(7, nb - 1)
                    if nb > 1:
                        rs_finish(nb - 2)
                else:
                    normalize(p - 1, nb)
                    normalize(p - 1 + 4, nb)
                    if nb == 0:  # weave remaining Q projections
                        for mb in range(4):
                            q_chain(mb, p)
                    else:
                        if p == 1:
                            oproj_chain(nb - 1, 0, 0, ppool)
                            oproj_chain(nb - 1, 0, 1, ppool)
                        elif p == 2:
                            oproj_chain(nb - 1, 1, 0, ppool)
                            oproj_chain(nb - 1, 1, 1, ppool)
                            oproj_chain(nb - 1, 2, 0, ppool)
                        else:
                            oproj_chain(nb - 1, 2, 1, ppool)
                            oproj_chain(nb - 1, 3, 0, ppool)
                            oproj_chain(nb - 1, 3, 1, ppool)
            if nb > 0:
                rs_issue(nb - 1)
        # tail: finish chunk 3
        normalize(3, 3)
        normalize(7, 3)
        for tb in range(4):
            for jh in range(2):
                # attention is done; alternate PSUM pools to pipeline
                oproj_chain(3, tb, jh, ppool if jh == 0 else avpool)
        rs_issue(3)
        rs_finish(2)
        rs_finish(3)

    _split_excess_waits(nc)
    return nc


def make_in_maps(x, wq, wk, wv, wo, bo):
    bf = ml_dtypes.bfloat16
    # local head h -> device slot (block h%4, half h//4): permuted head order
    hperm = [0, 4, 1, 5, 2, 6, 3, 7]
    dperm = np.concatenate([np.arange(64 * h, 64 * h + 64) for h in hperm])
    in_maps = []
    for c in range(NCORES):
        b, h2 = c // 2, c % 2
        wq_c = wq[HID * h2:HID * (h2 + 1)][dperm]  # [512, 1024] permuted rows
        wot_c = wo.T[HID * h2:HID * (h2 + 1)][dperm]  # [512, 1024] same perm
        in_maps.append({
            "xt": np.ascontiguousarray(x[b].T).astype(bf),
            "wqt": np.ascontiguousarray(wq_c.T).astype(bf),
            "wkt": np.ascontiguousarray(wk[P * h2:P * (h2 + 1)].T).astype(bf),
            "wvt": np.ascontiguousarray(wv[P * h2:P * (h2 + 1)].T).astype(bf),
            "wot": np.ascontiguousarray(wot_c).astype(bf),
            "bo_in": bo.astype(np.float32).reshape(1, D),
        })
    return in_maps


_CACHED_NC = None


def kernel(x, wq, wk, wv, wo, bo, _trace=False, _trace_kwargs=None):
    global _CACHED_NC
    from concourse.bass_utils import run_bass_kernel_spmd

    if _CACHED_NC is None:
        _CACHED_NC = build_nc()
    nc = _CACHED_NC

    in_maps = make_in_maps(
        np.asarray(x, np.float32), np.asarray(wq, np.float32),
        np.asarray(wk, np.float32), np.asarray(wv, np.float32),
        np.asarray(wo, np.float32), np.asarray(bo, np.float32))

    res = run_bass_kernel_spmd(
        nc, in_maps, core_ids=list(range(NCORES)),
        trace=_trace, **(_trace_kwargs or {}))

    out = np.empty((B, N, D), np.float32)
    for b in range(B):
        for h2 in range(2):
            r = res.results[2 * b + h2]["out_p"]
            for nb in range(NQB):
                out[b, nb * 512 + h2 * OWN: nb * 512 + (h2 + 1) * OWN] = \
                    r[nb * 512: nb * 512 + OWN]
    if _trace:
        kernel._last_results = res
    return out


# revision 11
# speedup vs baseline: 1.6516x; 1.1720x over previous
"""GQA attention kernel for 8 TRN2 NeuronCores (v2).

Problem: x[4,2048,1024], 16 Q heads / 4 KV heads, head_dim 64 (torch-Linear
style projections, softmax(QK^T/8)V, output projection + bias).

Sharding: core c handles (batch b = c//2, half h2 = c%2) where a half is
2 KV heads = 8 Q heads = 512 hidden dims. Per 512-token chunk, each core
computes the partial output projection over its 512 hidden dims (bf16,
pre-bias); the pair ReduceScatters it so each member ends up with the final
sum for a disjoint 256-token quarter, adds the bias, and writes those rows
to out_p. The program is identical on all cores (which quarter a core gets
falls out of its replica-group rank); the host stitches by core parity.

Attention inner loop (per query chunk nb, per head-pair (kv0 head, kv1
head)): the two K=64 QK matmuls sit at SBUF partition bases 0/64, so they
auto-derive tile_position (0,0)/(64,0) and run concurrently in disjoint PE
row groups; their [128,512] score tiles land in adjacent PSUM banks and are
exp'd by ONE FD=1024 ACTIVATE (ACT is the bottleneck engine; its ~300-cycle
fixed overhead is amortized). V is augmented with ones columns so the AV
matmul also produces softmax denominators broadcast across 64 partitions
for free. av tiles are copied to SBUF immediately (frees PSUM early) and
the expensive DVE reciprocals (8 cyc/elem) are deferred and woven between
later pairs so they never stall the QK->exp->AV stream.
"""

import sys
import numpy as np
from contextlib import ExitStack

sys.path.insert(0, "/opt/trn_rl_repo")

import ml_dtypes

from concourse import bass, tile, mybir


# ---------------------------------------------------------------------------
# This walrus build encodes at most 1-2 sync waits per instruction; the stock
# TileContext tail drain packs one wait per live proc onto a single Drain and
# fails codegen ("Too many sync wait commands"). Spread the waits over SP nop
# carriers instead.
def _patched_drain_and_barrier(self, tick_clock, wait_clock):
    from concourse.vector_clock import ScopedClock, VectorClock

    nc = self.nc
    gc = tick_clock.global_clock
    n = len(gc)
    for proc in range(n):
        t = gc[proc]
        if t <= 0:
            continue
        carrier = nc.sync.nop(nofuse=True)
        req = VectorClock([t if i == proc else 0 for i in range(n)])
        wait_clock.add_sem_waits(carrier.ins, ScopedClock({None: req}))
    nc.sync.drain()
    nc.all_engine_barrier()
    assert self.sems is not None
    popped = nc._tile_sem_poison_stack.pop()
    assert popped is self._sem_poison
    nc.clear_and_free_semaphores(list(self.sems.allocated().values()))
    nc.all_engine_barrier()


tile.TileContext._drain_and_barrier = _patched_drain_and_barrier


def _split_excess_waits(nc, max_waits=1):
    """Hoist all but one sync wait per instruction onto dedicated
    EventSemaphore carriers placed immediately before it on the same engine
    (same blocking semantics, one wait per encoded instruction)."""
    n_new = 0
    for bb in nc.main_func.blocks:
        il = list(bb.instructions)
        out = []
        changed = False
        for ins in il:
            si = ins.sync_info
            if si is not None:
                w = list(si.on_wait)
                if len(w) > max_waits:
                    for extra in w[max_waits:]:
                        ev = mybir.InstEventSemaphore(
                            name=f"{ins.name}-wsp{n_new}", engine=ins.engine)
                        n_new += 1
                        ev.sync_info = type(si)(on_wait=[extra], on_update=[])
                        nc.register_instruction(ev, overwrite=True)
                        out.append(ev)
                    si.on_wait = w[:max_waits]
                    changed = True
            out.append(ins)
        if changed:
            bb.instructions = out
# ---------------------------------------------------------------------------

B, N, D = 4, 2048, 1024
DH = 64  # head dim
HID = 512  # hidden dims per core (8 q heads)
NCORES = 8
P = 128
SCALE = DH ** -0.5
BF16 = mybir.dt.bfloat16
F32 = mybir.dt.float32

NB = N // P  # 16 key blocks of 128
NK = D // P  # 8 contraction chunks of 128
NQB = 4  # n query chunks of 512
VW = 256  # v chunk width: [64 v_kv0 | 64 ones | 64 v_kv1 | 64 ones]
OWN = 256  # tokens of each 512-chunk this core ends up with after RS

RG = [[0, 1], [2, 3], [4, 5], [6, 7]]


def build_nc():
    nc = bass.Bass(target_bir_lowering=False, debug=False, num_devices=NCORES)

    xt = nc.declare_dram_parameter("xt", [D, N], BF16, isOutput=False)
    wqt = nc.declare_dram_parameter("wqt", [D, HID], BF16, isOutput=False)
    wkt = nc.declare_dram_parameter("wkt", [D, P], BF16, isOutput=False)
    wvt = nc.declare_dram_parameter("wvt", [D, P], BF16, isOutput=False)
    wot = nc.declare_dram_parameter("wot", [HID, D], BF16, isOutput=False)
    bo_in = nc.declare_dram_parameter("bo_in", [1, D], F32, isOutput=False)
    out_p = nc.declare_dram_parameter("out_p", [N, D], F32, isOutput=True)

    # per-chunk partial o-proj (bf16, pre-bias) and its pair ReduceScatter
    po = [nc.dram_tensor(f"po{k}", [512, D], BF16) for k in range(NQB)]
    rs = [nc.dram_tensor(f"rs{k}", [OWN, D], BF16) for k in range(NQB)]

    with tile.TileContext(nc) as tc, ExitStack() as ctx:
        const = ctx.enter_context(tc.tile_pool(name="const", bufs=1))
        work = ctx.enter_context(tc.tile_pool(name="work", bufs=1))
        # PSUM: st 2x[128,1024] (4 banks) + av 3 + proj 1 = 8 banks
        stpool = ctx.enter_context(tc.tile_pool(name="stp", bufs=2, space="PSUM"))
        avpool = ctx.enter_context(tc.tile_pool(name="avp", bufs=3, space="PSUM"))
        ppool = ctx.enter_context(tc.tile_pool(name="ppool", bufs=1, space="PSUM"))
        ptpool = ctx.enter_context(tc.tile_pool(name="ptp", bufs=3))
        avsb = ctx.enter_context(tc.tile_pool(name="avsb", bufs=8))
        rpool = ctx.enter_context(tc.tile_pool(name="rp", bufs=3))
        popool = ctx.enter_context(tc.tile_pool(name="pop", bufs=3))
        finp = ctx.enter_context(tc.tile_pool(name="finp", bufs=2))

        # ---- load inputs -------------------------------------------------
        xt_sb = const.tile([P, NK * N], BF16)
        wkt_sb = const.tile([P, NK * P], BF16)
        for kc in range(NK):
            nc.sync.dma_start(out=wkt_sb[:, kc * P:(kc + 1) * P],
                              in_=wkt[kc * P:(kc + 1) * P, :])
            nc.sync.dma_start(out=xt_sb[:, kc * N:(kc + 1) * N],
                              in_=xt[kc * P:(kc + 1) * P, :])
        wqt_sb = const.tile([P, NK * HID], BF16)
        wvt_sb = const.tile([P, NK * P], BF16)
        for kc in range(NK):
            nc.sync.dma_start(out=wqt_sb[:, kc * HID:(kc + 1) * HID],
                              in_=wqt[kc * P:(kc + 1) * P, :])
            nc.sync.dma_start(out=wvt_sb[:, kc * P:(kc + 1) * P],
                              in_=wvt[kc * P:(kc + 1) * P, :])
        wot_sb = const.tile([P, 4 * D], BF16)
        for ic in range(4):
            nc.sync.dma_start(out=wot_sb[:, ic * D:(ic + 1) * D],
                              in_=wot[ic * P:(ic + 1) * P, :])
        bo_row = const.tile([1, D], F32)
        nc.sync.dma_start(out=bo_row[:], in_=bo_in[0:1, :])
        ones_row = const.tile([1, P], F32)
        nc.vector.memset(ones_row[:], 1.0)
        # partition-broadcast of the bias row via PE outer product
        bo_bc = const.tile([P, D], F32)
        for jh in range(2):
            bps = ppool.tile([P, 512], F32, tag="proj")
            nc.tensor.matmul(bps[:], lhsT=ones_row[:, 0:P],
                             rhs=bo_row[:, jh * 512:(jh + 1) * 512],
                             start=True, stop=True)
            nc.vector.tensor_copy(bo_bc[:, jh * 512:(jh + 1) * 512], bps[:])

        # ---- projections -------------------------------------------------
        # q^T [512, 2048] as 4 partition-blocks; k^T [128, 2048]; v natural.
        qt_sb = work.tile([P, 4 * N], BF16, tag="qt")
        kt_sb = work.tile([P, N], BF16, tag="kt")
        v_sb = work.tile([P, NB * VW], BF16, tag="v")
        nc.vector.memset(v_sb[:], 1.0)  # ones columns survive the copies

        def k_chain(nb):
            ps = avpool.tile([P, 512], F32, tag="av")
            for kc in range(NK):
                nc.tensor.matmul(
                    ps[:],
                    lhsT=wkt_sb[:, kc * P:(kc + 1) * P],
                    rhs=xt_sb[:, kc * N + nb * 512: kc * N + (nb + 1) * 512],
                    start=(kc == 0), stop=(kc == NK - 1),
                )
            nc.vector.tensor_copy(kt_sb[:, nb * 512:(nb + 1) * 512], ps[:])

        def v_chain(mb):
            ps = avpool.tile([P, 512], F32, tag="av")
            for kc in range(NK):
                nc.tensor.matmul(
                    ps[:, 0:P],
                    lhsT=xt_sb[:, kc * N + mb * P: kc * N + (mb + 1) * P],
                    rhs=wvt_sb[:, kc * P:(kc + 1) * P],
                    start=(kc == 0), stop=(kc == NK - 1),
                )
            nc.vector.tensor_copy(v_sb[:, mb * VW: mb * VW + 64], ps[:, 0:64])
            nc.vector.tensor_copy(v_sb[:, mb * VW + 128: mb * VW + 192],
                                  ps[:, 64:128])

        def q_chain(mb, nb):
            ps = ppool.tile([P, 512], F32, tag="proj")
            for kc in range(NK):
                nc.tensor.matmul(
                    ps[:],
                    lhsT=wqt_sb[:, kc * HID + mb * P: kc * HID + (mb + 1) * P],
                    rhs=xt_sb[:, kc * N + nb * 512: kc * N + (nb + 1) * 512],
                    start=(kc == 0), stop=(kc == NK - 1),
                )
            nc.vector.tensor_copy(
                qt_sb[:, mb * N + nb * 512: mb * N + (nb + 1) * 512], ps[:])

        for nb in range(NQB):
            k_chain(nb)
        for mb in range(NB):
            v_chain(mb)
        for mb in range(4):
            q_chain(mb, 0)

        # ---- attention ---------------------------------------------------
        # hidden^T [512, 2048] bf16, normalized attention outputs.
        # head h lives in q/hid block h%4 at partition half 64*(h//4), which
        # equals its kv head's half in kt (host-reordered weights).
        hid_sb = work.tile([P, 4 * N], BF16, tag="hid")

        avs_tiles = {}  # (h, nb) -> SBUF av tile [128,512] f32
        filler_q = []  # single-instruction closures woven into the mc loop
        filler_slots = [1]

        def pop_fillers(force_all=False):
            if force_all:
                n = len(filler_q)
            else:
                slots = max(filler_slots[0], 1)
                n = min(-(-len(filler_q) // slots), 3)
                filler_slots[0] -= 1
            for _ in range(n):
                filler_q.pop(0)()

        def attn_pair(p, nb, weave=False):
            """Heads hA=p (kv0, partitions 0:64) and hB=p+4 (kv1, 64:128)."""
            avA = avpool.tile([P, 512], F32, tag="av")
            avB = avpool.tile([P, 512], F32, tag="av")
            for mc in range(NB):
                st = stpool.tile([P, 1024], F32, tag="st")
                nc.tensor.matmul(
                    st[:, 0:512],
                    lhsT=kt_sb[0:64, mc * P:(mc + 1) * P],
                    rhs=qt_sb[0:64, p * N + nb * 512: p * N + (nb + 1) * 512],
                    start=True, stop=True,
                )
                nc.tensor.matmul(
                    st[:, 512:1024],
                    lhsT=kt_sb[64:128, mc * P:(mc + 1) * P],
                    rhs=qt_sb[64:128, p * N + nb * 512: p * N + (nb + 1) * 512],
                    start=True, stop=True,
                )
                pt = ptpool.tile([P, 1024], BF16, tag="pt")
                nc.scalar.activation(pt[:], st[:],
                                     mybir.ActivationFunctionType.Exp,
                                     scale=SCALE)
                nc.tensor.matmul(
                    avA[:],
                    lhsT=v_sb[:, mc * VW: mc * VW + P],
                    rhs=pt[:, 0:512],
                    start=(mc == 0), stop=(mc == NB - 1),
                )
                nc.tensor.matmul(
                    avB[:],
                    lhsT=v_sb[:, mc * VW + P: mc * VW + 2 * P],
                    rhs=pt[:, 512:1024],
                    start=(mc == 0), stop=(mc == NB - 1),
                )
                if weave and filler_q:
                    pop_fillers()
            sA = avsb.tile([P, 512], F32, tag="avs")
            sB = avsb.tile([P, 512], F32, tag="avs")
            nc.vector.tensor_copy(sA[:], avA[:])
            nc.vector.tensor_copy(sB[:], avB[:])
            avs_tiles[(p, nb)] = sA
            avs_tiles[(p + 4, nb)] = sB

        def normalize(h, nb):
            """hid[h-half, block h%4, nb chunk] = av_out * 1/den."""
            s = avs_tiles.pop((h, nb))
            hp = 64 * (h // 4)
            hb = h % 4
            rt = rpool.tile([64, 512], F32, tag="rt")
            nc.vector.reciprocal(rt[:], s[64:128, :])
            nc.vector.tensor_tensor(
                out=hid_sb[hp:hp + 64,
                           hb * N + nb * 512: hb * N + (nb + 1) * 512],
                in0=s[0:64, :], in1=rt[:],
                op=mybir.AluOpType.mult,
            )

        def oproj_mm(nb, tb, jh, ic, state, psum_pool=None):
            """One matmul of the partial o-proj chain for tokens
            [nb*512 + tb*128, +128), out dims [jh*512, +512); the last one
            also casts the psum to bf16 and ships it to the po buffer."""
            pool = psum_pool if psum_pool is not None else ppool
            if ic == 0:
                state['ps'] = pool.tile(
                    [P, 512], F32,
                    tag=("proj" if pool is ppool else "av"), name="ops")
            ps = state['ps']
            nc.tensor.matmul(
                ps[:],
                lhsT=hid_sb[:, ic * N + nb * 512 + tb * P:
                            ic * N + nb * 512 + (tb + 1) * P],
                rhs=wot_sb[:, ic * D + jh * 512: ic * D + (jh + 1) * 512],
                start=(ic == 0), stop=(ic == 3),
            )
            if ic == 3:
                pob = popool.tile([P, 512], BF16, tag="po")
                nc.vector.tensor_copy(pob[:], ps[:])
                nc.sync.dma_start(
                    out=po[nb][tb * P:(tb + 1) * P, jh * 512:(jh + 1) * 512],
                    in_=pob[:])

        def push_oproj_fillers(nb):
            for tb in range(4):
                for jh in range(2):
                    state = {}
                    for ic in range(4):
                        filler_q.append(
                            lambda nb=nb, tb=tb, jh=jh, ic=ic, state=state:
                            oproj_mm(nb, tb, jh, ic, state))

        def push_q_fillers(nb):
            for mb in range(4):
                state = {}

                def q_mm(kc, mb=mb, nb=nb, state=state):
                    if kc == 0:
                        state['ps'] = ppool.tile([P, 512], F32, tag="proj",
                                                 name="qps")
                    ps = state['ps']
                    nc.tensor.matmul(
                        ps[:],
                        lhsT=wqt_sb[:, kc * HID + mb * P:
                                    kc * HID + (mb + 1) * P],
                        rhs=xt_sb[:, kc * N + nb * 512:
                                  kc * N + (nb + 1) * 512],
                        start=(kc == 0), stop=(kc == NK - 1),
                    )
                    if kc == NK - 1:
                        nc.vector.tensor_copy(
                            qt_sb[:, mb * N + nb * 512:
                                  mb * N + (nb + 1) * 512], ps[:])
                for kc in range(NK):
                    filler_q.append(lambda kc=kc, f=q_mm: f(kc))

        def rs_issue(nb, half=None):
            if half is None:
                r0, r1 = 0, 512
            else:
                r0, r1 = half * OWN, half * OWN + OWN
            nc.gpsimd.collective_compute(
                "ReduceScatter", mybir.AluOpType.add,
                replica_groups=RG,
                ins=[po[nb][r0:r1, :].opt()],
                outs=[rs[nb][r0 // 2:r1 // 2, :].opt()],
            )

        def rs_finish(nb):
            """Read back our 256-token quarter, add bias, write out_p."""
            for tb in range(2):
                rsb = finp.tile([P, D], BF16, tag="rsb")
                nc.sync.dma_start(out=rsb[:], in_=rs[nb][tb * P:(tb + 1) * P, :])
                ot = finp.tile([P, D], F32, tag="ot")
                nc.vector.tensor_tensor(out=ot[:], in0=rsb[:], in1=bo_bc[:],
                                        op=mybir.AluOpType.add)
                r0 = nb * 512 + tb * P
                nc.sync.dma_start(out=out_p[r0:r0 + P, :], in_=ot[:])

        for nb in range(NQB):
            attn_pair(0, nb)
            if nb > 0:
                normalize(3, nb - 1)
                normalize(7, nb - 1)
            if nb > 1:
                rs_finish(nb - 2)
            if nb == 0:  # remaining Q projections become fillers
                for nbq in range(1, NQB):
                    push_q_fillers(nbq)
                filler_slots[0] = 3 * NB
                first_weave = 1
            else:
                push_oproj_fillers(nb - 1)
                # delay popping one pair so normalize(3/7, nb-1) finishes
                filler_slots[0] = 2 * NB
                first_weave = 2
            for p in range(1, 4):
                attn_pair(p, nb, weave=(p >= first_weave))
                normalize(p - 1, nb)
                normalize(p - 1 + 4, nb)
            pop_fillers(force_all=True)
            if nb > 0:
                rs_issue(nb - 1)
        # tail: finish chunk 3
        normalize(3, 3)
        normalize(7, 3)
        for tb in range(4):
            for jh in range(2):
                # attention is done; alternate PSUM pools to pipeline
                state = {}
                for ic in range(4):
                    oproj_mm(3, tb, jh, ic, state,
                             psum_pool=(ppool if jh == 0 else avpool))
        rs_issue(3)
        rs_finish(2)
        rs_finish(3)

    _split_excess_waits(nc)
    return nc


def make_in_maps(x, wq, wk, wv, wo, bo):
    bf = ml_dtypes.bfloat16
    # local head h -> device slot (block h%4, half h//4): permuted head order
    hperm = [0, 4, 1, 5, 2, 6, 3, 7]
    dperm = np.concatenate([np.arange(64 * h, 64 * h + 64) for h in hperm])
    in_maps = []
    for c in range(NCORES):
        b, h2 = c // 2, c % 2
        wq_c = wq[HID * h2:HID * (h2 + 1)][dperm]  # [512, 1024] permuted rows
        wot_c = wo.T[HID * h2:HID * (h2 + 1)][dperm]  # [512, 1024] same perm
        in_maps.append({
            "xt": np.ascontiguousarray(x[b].T).astype(bf),
            "wqt": np.ascontiguousarray(wq_c.T).astype(bf),
            "wkt": np.ascontiguousarray(wk[P * h2:P * (h2 + 1)].T).astype(bf),
            "wvt": np.ascontiguousarray(wv[P * h2:P * (h2 + 1)].T).astype(bf),
            "wot": np.ascontiguousarray(wot_c).astype(bf),
            "bo_in": bo.astype(np.float32).reshape(1, D),
        })
    return in_maps


_CACHED_NC = None


def kernel(x, wq, wk, wv, wo, bo, _trace=False, _trace_kwargs=None):
    global _CACHED_NC
    from concourse.bass_utils import run_bass_kernel_spmd

    if _CACHED_NC is None:
        _CACHED_NC = build_nc()
    nc = _CACHED_NC

    in_maps = make_in_maps(
        np.asarray(x, np.float32), np.asarray(wq, np.float32),
        np.asarray(wk, np.float32), np.asarray(wv, np.float32),
        np.asarray(wo, np.float32), np.asarray(bo, np.float32))

    res = run_bass_kernel_spmd(
        nc, in_maps, core_ids=list(range(NCORES)),
        trace=_trace, **(_trace_kwargs or {}))

    out = np.empty((B, N, D), np.float32)
    for b in range(B):
        for h2 in range(2):
            r = res.results[2 * b + h2]["out_p"]
            for nb in range(NQB):
                out[b, nb * 512 + h2 * OWN: nb * 512 + (h2 + 1) * OWN] = \
                    r[nb * 512: nb * 512 + OWN]
    if _trace:
        kernel._last_results = res
    return out


# revision 28
# speedup vs baseline: 1.8451x; 1.1172x over previous
"""GQA attention kernel for 8 TRN2 NeuronCores (v2).

Problem: x[4,2048,1024], 16 Q heads / 4 KV heads, head_dim 64 (torch-Linear
style projections, softmax(QK^T/8)V, output projection + bias).

Sharding: core c handles (batch b = c//2, half h2 = c%2) where a half is
2 KV heads = 8 Q heads = 512 hidden dims. Per 512-token chunk, each core
computes the partial output projection over its 512 hidden dims (bf16,
pre-bias); the pair ReduceScatters it so each member ends up with the final
sum for a disjoint 256-token quarter, adds the bias, and writes those rows
to out_p. The program is identical on all cores (which quarter a core gets
falls out of its replica-group rank); the host stitches by core parity.

Attention inner loop (per query chunk nb, per head-pair (kv0 head, kv1
head)): the two K=64 QK matmuls sit at SBUF partition bases 0/64, so they
auto-derive tile_position (0,0)/(64,0) and run concurrently in disjoint PE
row groups; their [128,512] score tiles land in adjacent PSUM banks and are
exp'd by ONE FD=1024 ACTIVATE (ACT is the bottleneck engine; its ~300-cycle
fixed overhead is amortized). V is augmented with ones columns so the AV
matmul also produces softmax denominators broadcast across 64 partitions
for free. av tiles are copied to SBUF immediately (frees PSUM early) and
the expensive DVE reciprocals (8 cyc/elem) are deferred and woven between
later pairs so they never stall the QK->exp->AV stream.
"""

import sys
import numpy as np
from contextlib import ExitStack

sys.path.insert(0, "/opt/trn_rl_repo")

import ml_dtypes

from concourse import bass, tile, mybir


# ---------------------------------------------------------------------------
# This walrus build encodes at most 1-2 sync waits per instruction; the stock
# TileContext tail drain packs one wait per live proc onto a single Drain and
# fails codegen ("Too many sync wait commands"). Spread the waits over SP nop
# carriers instead.
def _patched_drain_and_barrier(self, tick_clock, wait_clock):
    from concourse.vector_clock import ScopedClock, VectorClock

    nc = self.nc
    gc = tick_clock.global_clock
    n = len(gc)
    for proc in range(n):
        t = gc[proc]
        if t <= 0:
            continue
        carrier = nc.sync.nop(nofuse=True)
        req = VectorClock([t if i == proc else 0 for i in range(n)])
        wait_clock.add_sem_waits(carrier.ins, ScopedClock({None: req}))
    nc.sync.drain()
    nc.all_engine_barrier()
    assert self.sems is not None
    popped = nc._tile_sem_poison_stack.pop()
    assert popped is self._sem_poison
    nc.clear_and_free_semaphores(list(self.sems.allocated().values()))
    nc.all_engine_barrier()


tile.TileContext._drain_and_barrier = _patched_drain_and_barrier


def _split_excess_waits(nc, max_waits=1):
    """Hoist all but one sync wait per instruction onto dedicated
    EventSemaphore carriers placed immediately before it on the same engine
    (same blocking semantics, one wait per encoded instruction)."""
    n_new = 0
    for bb in nc.main_func.blocks:
        il = list(bb.instructions)
        out = []
        changed = False
        for ins in il:
            si = ins.sync_info
            if si is not None:
                w = list(si.on_wait)
                if len(w) > max_waits:
                    for extra in w[max_waits:]:
                        ev = mybir.InstEventSemaphore(
                            name=f"{ins.name}-wsp{n_new}", engine=ins.engine)
                        n_new += 1
                        ev.sync_info = type(si)(on_wait=[extra], on_update=[])
                        nc.register_instruction(ev, overwrite=True)
                        out.append(ev)
                    si.on_wait = w[:max_waits]
                    changed = True
            out.append(ins)
        if changed:
            bb.instructions = out
# ---------------------------------------------------------------------------

B, N, D = 4, 2048, 1024
DH = 64  # head dim
HID = 512  # hidden dims per core (8 q heads)
NCORES = 8
P = 128
SCALE = DH ** -0.5
BF16 = mybir.dt.bfloat16
F32 = mybir.dt.float32

NB = N // P  # 16 key blocks of 128
NK = D // P  # 8 contraction chunks of 128
NQB = 4  # n query chunks of 512
VW = 256  # v chunk width: [64 v_kv0 | 64 ones | 64 v_kv1 | 64 ones]
OWN = 256  # tokens of each 512-chunk this core ends up with after RS

RG = [[0, 1], [2, 3], [4, 5], [6, 7]]


def build_nc():
    nc = bass.Bass(target_bir_lowering=False, debug=False, num_devices=NCORES)

    xt = nc.declare_dram_parameter("xt", [D, N], BF16, isOutput=False)
    wqt = nc.declare_dram_parameter("wqt", [D, HID], BF16, isOutput=False)
    wkt = nc.declare_dram_parameter("wkt", [D, P], BF16, isOutput=False)
    wvt = nc.declare_dram_parameter("wvt", [D, P], BF16, isOutput=False)
    wot = nc.declare_dram_parameter("wot", [HID, D], BF16, isOutput=False)
    bo_in = nc.declare_dram_parameter("bo_in", [1, D], F32, isOutput=False)
    out_p = nc.declare_dram_parameter("out_p", [N, D], F32, isOutput=True)

    # per-chunk partial o-proj (bf16, pre-bias) and its pair ReduceScatter
    po = [nc.dram_tensor(f"po{k}", [512, D], BF16) for k in range(NQB)]
    rs = [nc.dram_tensor(f"rs{k}", [OWN, D], BF16) for k in range(NQB)]
    # bounce buffer for the packed softmax reciprocals (DRAM so the
    # partition-replicating read-back can use a stride-0 outer dim)
    rd = [nc.dram_tensor(f"rd{k}", [32, 128], F32) for k in range(NQB)]

    with tile.TileContext(nc) as tc, ExitStack() as ctx:
        const = ctx.enter_context(tc.tile_pool(name="const", bufs=1))
        work = ctx.enter_context(tc.tile_pool(name="work", bufs=1))
        # PSUM: st 2x[128,1024] (4 banks) + av 3 + proj 1 = 8 banks
        stpool = ctx.enter_context(tc.tile_pool(name="stp", bufs=2, space="PSUM"))
        avpool = ctx.enter_context(tc.tile_pool(name="avp", bufs=3, space="PSUM"))
        ppool = ctx.enter_context(tc.tile_pool(name="ppool", bufs=1, space="PSUM"))
        ptpool = ctx.enter_context(tc.tile_pool(name="ptp", bufs=3))
        avsb = ctx.enter_context(tc.tile_pool(name="avsb", bufs=12))
        rpool = ctx.enter_context(tc.tile_pool(name="rp", bufs=3))
        popool = ctx.enter_context(tc.tile_pool(name="pop", bufs=3))
        finp = ctx.enter_context(tc.tile_pool(name="finp", bufs=2))

        # ---- load inputs -------------------------------------------------
        xt_sb = const.tile([P, NK * N], BF16)
        wkt_sb = const.tile([P, NK * P], BF16)
        bo_row = const.tile([1, D], F32)
        nc.sync.dma_start(out=bo_row[:], in_=bo_in[0:1, :])
        for kc in range(NK):
            nc.sync.dma_start(out=wkt_sb[:, kc * P:(kc + 1) * P],
                              in_=wkt[kc * P:(kc + 1) * P, :])
            nc.sync.dma_start(out=xt_sb[:, kc * N:(kc + 1) * N],
                              in_=xt[kc * P:(kc + 1) * P, :])
        wqt_sb = const.tile([P, NK * HID], BF16)
        wvt_sb = const.tile([P, NK * P], BF16)
        for kc in range(NK):
            nc.sync.dma_start(out=wqt_sb[:, kc * HID:(kc + 1) * HID],
                              in_=wqt[kc * P:(kc + 1) * P, :])
            nc.sync.dma_start(out=wvt_sb[:, kc * P:(kc + 1) * P],
                              in_=wvt[kc * P:(kc + 1) * P, :])
        wot_sb = const.tile([P, 4 * D], BF16)
        for ic in range(4):
            nc.sync.dma_start(out=wot_sb[:, ic * D:(ic + 1) * D],
                              in_=wot[ic * P:(ic + 1) * P, :])
        ones_row = const.tile([1, P], F32)
        nc.vector.memset(ones_row[:], 1.0)
        bo_bc = const.tile([P, D], F32)

        # ---- projections -------------------------------------------------
        # q^T [512, 2048] as 4 partition-blocks; k^T [128, 2048]; v natural.
        qt_sb = work.tile([P, 4 * N], BF16, tag="qt")
        kt_sb = work.tile([P, N], BF16, tag="kt")
        v_sb = work.tile([P, NB * VW], BF16, tag="v")
        nc.vector.memset(v_sb[:], 1.0)  # ones columns survive the copies

        def k_chain(nb):
            ps = avpool.tile([P, 512], F32, tag="av")
            for kc in range(NK):
                nc.tensor.matmul(
                    ps[:],
                    lhsT=wkt_sb[:, kc * P:(kc + 1) * P],
                    rhs=xt_sb[:, kc * N + nb * 512: kc * N + (nb + 1) * 512],
                    start=(kc == 0), stop=(kc == NK - 1),
                )
            nc.vector.tensor_copy(kt_sb[:, nb * 512:(nb + 1) * 512], ps[:])

        def v_chain(mb):
            ps = avpool.tile([P, 512], F32, tag="av")
            for kc in range(NK):
                nc.tensor.matmul(
                    ps[:, 0:P],
                    lhsT=xt_sb[:, kc * N + mb * P: kc * N + (mb + 1) * P],
                    rhs=wvt_sb[:, kc * P:(kc + 1) * P],
                    start=(kc == 0), stop=(kc == NK - 1),
                )
            nc.vector.tensor_copy(v_sb[:, mb * VW: mb * VW + 64], ps[:, 0:64])
            nc.vector.tensor_copy(v_sb[:, mb * VW + 128: mb * VW + 192],
                                  ps[:, 64:128])

        def q_chain(mb, nb):
            ps = ppool.tile([P, 512], F32, tag="proj")
            for kc in range(NK):
                nc.tensor.matmul(
                    ps[:],
                    lhsT=wqt_sb[:, kc * HID + mb * P: kc * HID + (mb + 1) * P],
                    rhs=xt_sb[:, kc * N + nb * 512: kc * N + (nb + 1) * 512],
                    start=(kc == 0), stop=(kc == NK - 1),
                )
            nc.vector.tensor_copy(
                qt_sb[:, mb * N + nb * 512: mb * N + (nb + 1) * 512], ps[:])

        for nb in range(NQB):
            k_chain(nb)
        for mb in range(NB):
            v_chain(mb)
        for mb in range(4):
            q_chain(mb, 0)
        # partition-broadcast of the bias row via PE outer product (late so
        # it never blocks the projection stream at the PE FIFO head)
        for jh in range(2):
            bps = ppool.tile([P, 512], F32, tag="proj")
            nc.tensor.matmul(bps[:], lhsT=ones_row[:, 0:P],
                             rhs=bo_row[:, jh * 512:(jh + 1) * 512],
                             start=True, stop=True)
            nc.vector.tensor_copy(bo_bc[:, jh * 512:(jh + 1) * 512], bps[:])

        # ---- attention ---------------------------------------------------
        # hidden^T [512, 2048] bf16, normalized attention outputs.
        # head h lives in q/hid block h%4 at partition half 64*(h//4), which
        # equals its kv head's half in kt (host-reordered weights).
        hid_sb = work.tile([P, 4 * N], BF16, tag="hid")

        avs_tiles = {}  # (h, nb) -> SBUF av tile [128,512] f32
        dpk_tiles = {}  # nb -> [32,128] f32 packed softmax denominators
        filler_q = []  # single-instruction closures woven into the mc loop
        filler_slots = [1]

        def pop_fillers(force_all=False):
            if force_all:
                n = len(filler_q)
            else:
                slots = max(filler_slots[0], 1)
                n = min(-(-len(filler_q) // slots), 3)
                filler_slots[0] -= 1
            for _ in range(n):
                filler_q.pop(0)()

        def attn_pair(p, nb, weave=False):
            """Heads hA=p (kv0, partitions 0:64) and hB=p+4 (kv1, 64:128)."""
            avA = avpool.tile([P, 512], F32, tag="av")
            avB = avpool.tile([P, 512], F32, tag="av")
            for mc in range(NB):
                st = stpool.tile([P, 1024], F32, tag="st")
                nc.tensor.matmul(
                    st[:, 0:512],
                    lhsT=kt_sb[0:64, mc * P:(mc + 1) * P],
                    rhs=qt_sb[0:64, p * N + nb * 512: p * N + (nb + 1) * 512],
                    start=True, stop=True,
                )
                nc.tensor.matmul(
                    st[:, 512:1024],
                    lhsT=kt_sb[64:128, mc * P:(mc + 1) * P],
                    rhs=qt_sb[64:128, p * N + nb * 512: p * N + (nb + 1) * 512],
                    start=True, stop=True,
                )
                pt = ptpool.tile([P, 1024], BF16, tag="pt")
                nc.scalar.activation(pt[:], st[:],
                                     mybir.ActivationFunctionType.Exp,
                                     scale=SCALE)
                nc.tensor.matmul(
                    avA[:],
                    lhsT=v_sb[:, mc * VW: mc * VW + P],
                    rhs=pt[:, 0:512],
                    start=(mc == 0), stop=(mc == NB - 1),
                )
                nc.tensor.matmul(
                    avB[:],
                    lhsT=v_sb[:, mc * VW + P: mc * VW + 2 * P],
                    rhs=pt[:, 512:1024],
                    start=(mc == 0), stop=(mc == NB - 1),
                )
                if weave and filler_q:
                    pop_fillers()
            sA = avsb.tile([P, 512], F32, tag="avs")
            sB = avsb.tile([P, 512], F32, tag="avs")
            nc.vector.tensor_copy(sA[:], avA[:])
            nc.vector.tensor_copy(sB[:], avB[:])
            avs_tiles[(p, nb)] = sA
            avs_tiles[(p + 4, nb)] = sB
            # densify this pair's softmax denominators (row 64 carries them,
            # replicated x64): head h -> dpk rows [4h, 4h+4)
            dpk = dpk_tiles[nb]
            for h in (p, p + 4):
                s = avs_tiles[(h, nb)]
                nc.sync.dma_start(out=dpk[4 * h:4 * h + 4, 0:128],
                                  in_=s[64:65, 0:512])

        def normalize_chunk(nb):
            """One dense [32,128] reciprocal for the whole chunk, then DMA
            each head's row back replicated across 64 partitions and scale
            the attention outputs into hid."""
            dpk = dpk_tiles.pop(nb)
            rcp = rpool.tile([32, 128], F32, tag="rcp")
            nc.vector.reciprocal(rcp[:], dpk[:])
            nc.sync.dma_start(out=rd[nb][0:32, :], in_=rcp[:])
            for h in range(8):
                s = avs_tiles.pop((h, nb))
                rt = rpool.tile([64, 512], F32, tag="rt")
                nc.sync.dma_start(
                    out=rt[:],
                    in_=rd[nb][4 * h:4 * h + 4, :].flatten()[None]
                    .broadcast_to([64, 512]))
                hp = 64 * (h // 4)
                hb = h % 4
                nc.vector.tensor_tensor(
                    out=hid_sb[hp:hp + 64,
                               hb * N + nb * 512: hb * N + (nb + 1) * 512],
                    in0=s[0:64, :], in1=rt[:],
                    op=mybir.AluOpType.mult,
                )

        def oproj_mm(nb, tb, jh, ic, state, psum_pool=None):
            """One matmul of the partial o-proj chain for tokens
            [nb*512 + tb*128, +128), out dims [jh*512, +512); the last one
            also casts the psum to bf16 and ships it to the po buffer."""
            pool = psum_pool if psum_pool is not None else ppool
            if ic == 0:
                state['ps'] = pool.tile(
                    [P, 512], F32,
                    tag=("proj" if pool is ppool else "av"), name="ops")
            ps = state['ps']
            nc.tensor.matmul(
                ps[:],
                lhsT=hid_sb[:, ic * N + nb * 512 + tb * P:
                            ic * N + nb * 512 + (tb + 1) * P],
                rhs=wot_sb[:, ic * D + jh * 512: ic * D + (jh + 1) * 512],
                start=(ic == 0), stop=(ic == 3),
            )
            if ic == 3:
                pob = popool.tile([P, 512], BF16, tag="po")
                nc.vector.tensor_copy(pob[:], ps[:])
                nc.sync.dma_start(
                    out=po[nb][tb * P:(tb + 1) * P, jh * 512:(jh + 1) * 512],
                    in_=pob[:])

        def push_oproj_fillers(nb):
            for tb in range(4):
                for jh in range(2):
                    state = {}
                    for ic in range(4):
                        filler_q.append(
                            lambda nb=nb, tb=tb, jh=jh, ic=ic, state=state:
                            oproj_mm(nb, tb, jh, ic, state))

        def push_q_fillers(nb):
            for mb in range(4):
                state = {}

                def q_mm(kc, mb=mb, nb=nb, state=state):
                    if kc == 0:
                        state['ps'] = ppool.tile([P, 512], F32, tag="proj",
                                                 name="qps")
                    ps = state['ps']
                    nc.tensor.matmul(
                        ps[:],
                        lhsT=wqt_sb[:, kc * HID + mb * P:
                                    kc * HID + (mb + 1) * P],
                        rhs=xt_sb[:, kc * N + nb * 512:
                                  kc * N + (nb + 1) * 512],
                        start=(kc == 0), stop=(kc == NK - 1),
                    )
                    if kc == NK - 1:
                        nc.vector.tensor_copy(
                            qt_sb[:, mb * N + nb * 512:
                                  mb * N + (nb + 1) * 512], ps[:])
                for kc in range(NK):
                    filler_q.append(lambda kc=kc, f=q_mm: f(kc))

        def rs_issue(nb, half=None):
            if half is None:
                r0, r1 = 0, 512
            else:
                r0, r1 = half * OWN, half * OWN + OWN
            nc.gpsimd.collective_compute(
                "ReduceScatter", mybir.AluOpType.add,
                replica_groups=RG,
                ins=[po[nb][r0:r1, :].opt()],
                outs=[rs[nb][r0 // 2:r1 // 2, :].opt()],
            )

        def rs_finish(nb):
            """Read back our 256-token quarter, add bias, write out_p."""
            for tb in range(2):
                rsb = finp.tile([P, D], BF16, tag="rsb")
                nc.sync.dma_start(out=rsb[:], in_=rs[nb][tb * P:(tb + 1) * P, :])
                ot = finp.tile([P, D], F32, tag="ot")
                nc.vector.tensor_tensor(out=ot[:], in0=rsb[:], in1=bo_bc[:],
                                        op=mybir.AluOpType.add)
                r0 = nb * 512 + tb * P
                nc.sync.dma_start(out=out_p[r0:r0 + P, :], in_=ot[:])

        for nb in range(NQB):
            dpk_tiles[nb] = avsb.tile([32, 128], F32, tag="dpk", bufs=2,
                                      name="dpk")
            attn_pair(0, nb)
            if nb > 0:
                normalize_chunk(nb - 1)
            if nb > 1:
                rs_finish(nb - 2)
            if nb == 0:  # remaining Q projections become fillers
                for nbq in range(1, NQB):
                    push_q_fillers(nbq)
                filler_slots[0] = 3 * NB
                first_weave = 1
            else:
                push_oproj_fillers(nb - 1)
                # delay popping one pair so normalize_chunk(nb-1) finishes
                filler_slots[0] = 2 * NB
                first_weave = 2
            for p in range(1, 4):
                attn_pair(p, nb, weave=(p >= first_weave))
            pop_fillers(force_all=True)
            if nb > 0:
                rs_issue(nb - 1)
        # tail: finish chunk 3
        normalize_chunk(3)
        for tb in range(4):
            for jh in range(2):
                # attention is done; alternate PSUM pools to pipeline
                state = {}
                for ic in range(4):
                    oproj_mm(3, tb, jh, ic, state,
                             psum_pool=(ppool if jh == 0 else avpool))
        rs_issue(3)
        rs_finish(2)
        rs_finish(3)

    _split_excess_waits(nc)
    return nc


def make_in_maps(x, wq, wk, wv, wo, bo):
    bf = ml_dtypes.bfloat16
    # local head h -> device slot (block h%4, half h//4): permuted head order
    hperm = [0, 4, 1, 5, 2, 6, 3, 7]
    dperm = np.concatenate([np.arange(64 * h, 64 * h + 64) for h in hperm])
    in_maps = []
    for c in range(NCORES):
        b, h2 = c // 2, c % 2
        wq_c = wq[HID * h2:HID * (h2 + 1)][dperm]  # [512, 1024] permuted rows
        wot_c = wo.T[HID * h2:HID * (h2 + 1)][dperm]  # [512, 1024] same perm
        in_maps.append({
            "xt": np.ascontiguousarray(x[b].T).astype(bf),
            "wqt": np.ascontiguousarray(wq_c.T).astype(bf),
            "wkt": np.ascontiguousarray(wk[P * h2:P * (h2 + 1)].T).astype(bf),
            "wvt": np.ascontiguousarray(wv[P * h2:P * (h2 + 1)].T).astype(bf),
            "wot": np.ascontiguousarray(wot_c).astype(bf),
            "bo_in": bo.astype(np.float32).reshape(1, D),
        })
    return in_maps


_CACHED_NC = None


def kernel(x, wq, wk, wv, wo, bo, _trace=False, _trace_kwargs=None):
    global _CACHED_NC
    from concourse.bass_utils import run_bass_kernel_spmd

    if _CACHED_NC is None:
        _CACHED_NC = build_nc()
    nc = _CACHED_NC

    in_maps = make_in_maps(
        np.asarray(x, np.float32), np.asarray(wq, np.float32),
        np.asarray(wk, np.float32), np.asarray(wv, np.float32),
        np.asarray(wo, np.float32), np.asarray(bo, np.float32))

    res = run_bass_kernel_spmd(
        nc, in_maps, core_ids=list(range(NCORES)),
        trace=_trace, **(_trace_kwargs or {}))

    out = np.empty((B, N, D), np.float32)
    for b in range(B):
        for h2 in range(2):
            r = res.results[2 * b + h2]["out_p"]
            for nb in range(NQB):
                out[b, nb * 512 + h2 * OWN: nb * 512 + (h2 + 1) * OWN] = \
                    r[nb * 512: nb * 512 + OWN]
    if _trace:
        kernel._last_results = res
    return out


# revision 33
# speedup vs baseline: 1.8985x; 1.0289x over previous
"""GQA attention kernel for 8 TRN2 NeuronCores (v2).

Problem: x[4,2048,1024], 16 Q heads / 4 KV heads, head_dim 64 (torch-Linear
style projections, softmax(QK^T/8)V, output projection + bias).

Sharding: core c handles (batch b = c//2, half h2 = c%2) where a half is
2 KV heads = 8 Q heads = 512 hidden dims. Per 512-token chunk, each core
computes the partial output projection over its 512 hidden dims (bf16,
pre-bias); the pair ReduceScatters it so each member ends up with the final
sum for a disjoint 256-token quarter, adds the bias, and writes those rows
to out_p. The program is identical on all cores (which quarter a core gets
falls out of its replica-group rank); the host stitches by core parity.

Attention inner loop (per query chunk nb, per head-pair (kv0 head, kv1
head)): the two K=64 QK matmuls sit at SBUF partition bases 0/64, so they
auto-derive tile_position (0,0)/(64,0) and run concurrently in disjoint PE
row groups; their [128,512] score tiles land in adjacent PSUM banks and are
exp'd by ONE FD=1024 ACTIVATE (ACT is the bottleneck engine; its ~300-cycle
fixed overhead is amortized). V is augmented with ones columns so the AV
matmul also produces softmax denominators broadcast across 64 partitions
for free. av tiles are copied to SBUF immediately (frees PSUM early) and
the expensive DVE reciprocals (8 cyc/elem) are deferred and woven between
later pairs so they never stall the QK->exp->AV stream.
"""

import sys
import numpy as np
from contextlib import ExitStack

sys.path.insert(0, "/opt/trn_rl_repo")

import ml_dtypes

from concourse import bass, tile, mybir


# ---------------------------------------------------------------------------
# This walrus build encodes at most 1-2 sync waits per instruction; the stock
# TileContext tail drain packs one wait per live proc onto a single Drain and
# fails codegen ("Too many sync wait commands"). Spread the waits over SP nop
# carriers instead.
def _patched_drain_and_barrier(self, tick_clock, wait_clock):
    from concourse.vector_clock import ScopedClock, VectorClock

    nc = self.nc
    gc = tick_clock.global_clock
    n = len(gc)
    for proc in range(n):
        t = gc[proc]
        if t <= 0:
            continue
        carrier = nc.sync.nop(nofuse=True)
        req = VectorClock([t if i == proc else 0 for i in range(n)])
        wait_clock.add_sem_waits(carrier.ins, ScopedClock({None: req}))
    nc.sync.drain()
    nc.all_engine_barrier()
    assert self.sems is not None
    popped = nc._tile_sem_poison_stack.pop()
    assert popped is self._sem_poison
    nc.clear_and_free_semaphores(list(self.sems.allocated().values()))
    nc.all_engine_barrier()


tile.TileContext._drain_and_barrier = _patched_drain_and_barrier


def _split_excess_waits(nc, max_waits=1):
    """Hoist all but one sync wait per instruction onto dedicated
    EventSemaphore carriers placed immediately before it on the same engine
    (same blocking semantics, one wait per encoded instruction)."""
    n_new = 0
    for bb in nc.main_func.blocks:
        il = list(bb.instructions)
        out = []
        changed = False
        for ins in il:
            si = ins.sync_info
            if si is not None:
                w = list(si.on_wait)
                if len(w) > max_waits:
                    for extra in w[max_waits:]:
                        ev = mybir.InstEventSemaphore(
                            name=f"{ins.name}-wsp{n_new}", engine=ins.engine)
                        n_new += 1
                        ev.sync_info = type(si)(on_wait=[extra], on_update=[])
                        nc.register_instruction(ev, overwrite=True)
                        out.append(ev)
                    si.on_wait = w[:max_waits]
                    changed = True
            out.append(ins)
        if changed:
            bb.instructions = out
# ---------------------------------------------------------------------------

B, N, D = 4, 2048, 1024
DH = 64  # head dim
HID = 512  # hidden dims per core (8 q heads)
NCORES = 8
P = 128
SCALE = DH ** -0.5
BF16 = mybir.dt.bfloat16
F32 = mybir.dt.float32

NB = N // P  # 16 key blocks of 128
NK = D // P  # 8 contraction chunks of 128
NQB = 4  # n query chunks of 512
VW = 256  # v chunk width: [64 v_kv0 | 64 ones | 64 v_kv1 | 64 ones]
OWN = 256  # tokens of each 512-chunk this core ends up with after RS

RG = [[0, 1], [2, 3], [4, 5], [6, 7]]


def build_nc():
    nc = bass.Bass(target_bir_lowering=False, debug=False, num_devices=NCORES)

    xt = nc.declare_dram_parameter("xt", [D, N], BF16, isOutput=False)
    wqt = nc.declare_dram_parameter("wqt", [D, HID], BF16, isOutput=False)
    wkt = nc.declare_dram_parameter("wkt", [D, P], BF16, isOutput=False)
    wvt = nc.declare_dram_parameter("wvt", [D, P], BF16, isOutput=False)
    wot = nc.declare_dram_parameter("wot", [HID, D], BF16, isOutput=False)
    bo_in = nc.declare_dram_parameter("bo_in", [1, D], F32, isOutput=False)
    out_p = nc.declare_dram_parameter("out_p", [N, D], F32, isOutput=True)

    # per-chunk partial o-proj (bf16, pre-bias) and its pair ReduceScatter
    po = [nc.dram_tensor(f"po{k}", [512, D], BF16) for k in range(NQB)]
    rs = [nc.dram_tensor(f"rs{k}", [OWN, D], BF16) for k in range(NQB)]
    # bounce buffer for the packed softmax reciprocals (DRAM so the
    # partition-replicating read-back can use a stride-0 outer dim)
    rd = [nc.dram_tensor(f"rd{k}", [32, 128], F32) for k in range(NQB)]

    with tile.TileContext(nc) as tc, ExitStack() as ctx:
        const = ctx.enter_context(tc.tile_pool(name="const", bufs=1))
        work = ctx.enter_context(tc.tile_pool(name="work", bufs=1))
        # PSUM: st 2x[128,1024] (4 banks) + av 3 + proj 1 = 8 banks
        stpool = ctx.enter_context(tc.tile_pool(name="stp", bufs=2, space="PSUM"))
        avpool = ctx.enter_context(tc.tile_pool(name="avp", bufs=3, space="PSUM"))
        ppool = ctx.enter_context(tc.tile_pool(name="ppool", bufs=1, space="PSUM"))
        ptpool = ctx.enter_context(tc.tile_pool(name="ptp", bufs=3))
        avsb = ctx.enter_context(tc.tile_pool(name="avsb", bufs=12))
        rpool = ctx.enter_context(tc.tile_pool(name="rp", bufs=3))
        popool = ctx.enter_context(tc.tile_pool(name="pop", bufs=3))
        finp = ctx.enter_context(tc.tile_pool(name="finp", bufs=2))

        # ---- load inputs -------------------------------------------------
        xt_sb = const.tile([P, NK * N], BF16)
        wkt_sb = const.tile([P, NK * P], BF16)
        bo_row = const.tile([1, D], F32)
        nc.sync.dma_start(out=bo_row[:], in_=bo_in[0:1, :])
        for kc in range(NK):
            nc.sync.dma_start(out=wkt_sb[:, kc * P:(kc + 1) * P],
                              in_=wkt[kc * P:(kc + 1) * P, :])
            nc.sync.dma_start(out=xt_sb[:, kc * N:(kc + 1) * N],
                              in_=xt[kc * P:(kc + 1) * P, :])
        wqt_sb = const.tile([P, NK * HID], BF16)
        wvt_sb = const.tile([P, NK * P], BF16)
        for kc in range(NK):
            nc.sync.dma_start(out=wqt_sb[:, kc * HID:(kc + 1) * HID],
                              in_=wqt[kc * P:(kc + 1) * P, :])
            nc.sync.dma_start(out=wvt_sb[:, kc * P:(kc + 1) * P],
                              in_=wvt[kc * P:(kc + 1) * P, :])
        wot_sb = const.tile([P, 4 * D], BF16)
        for ic in range(4):
            nc.sync.dma_start(out=wot_sb[:, ic * D:(ic + 1) * D],
                              in_=wot[ic * P:(ic + 1) * P, :])
        ones_row = const.tile([1, P], F32)
        nc.vector.memset(ones_row[:], 1.0)
        bo_bc = const.tile([P, D], F32)

        # ---- projections -------------------------------------------------
        # q^T [512, 2048] as 4 partition-blocks; k^T [128, 2048]; v natural.
        qt_sb = work.tile([P, 4 * N], BF16, tag="qt")
        kt_sb = work.tile([P, N], BF16, tag="kt")
        v_sb = work.tile([P, NB * VW], BF16, tag="v")
        nc.vector.memset(v_sb[:], 1.0)  # ones columns survive the copies

        def k_chain(nb):
            ps = avpool.tile([P, 512], F32, tag="av")
            for kc in range(NK):
                nc.tensor.matmul(
                    ps[:],
                    lhsT=wkt_sb[:, kc * P:(kc + 1) * P],
                    rhs=xt_sb[:, kc * N + nb * 512: kc * N + (nb + 1) * 512],
                    start=(kc == 0), stop=(kc == NK - 1),
                )
            nc.vector.tensor_copy(kt_sb[:, nb * 512:(nb + 1) * 512], ps[:])

        def v_chain(mb):
            ps = avpool.tile([P, 512], F32, tag="av")
            for kc in range(NK):
                nc.tensor.matmul(
                    ps[:, 0:P],
                    lhsT=xt_sb[:, kc * N + mb * P: kc * N + (mb + 1) * P],
                    rhs=wvt_sb[:, kc * P:(kc + 1) * P],
                    start=(kc == 0), stop=(kc == NK - 1),
                )
            nc.vector.tensor_copy(v_sb[:, mb * VW: mb * VW + 64], ps[:, 0:64])
            nc.vector.tensor_copy(v_sb[:, mb * VW + 128: mb * VW + 192],
                                  ps[:, 64:128])

        def q_chain(mb, nb):
            ps = ppool.tile([P, 512], F32, tag="proj")
            for kc in range(NK):
                nc.tensor.matmul(
                    ps[:],
                    lhsT=wqt_sb[:, kc * HID + mb * P: kc * HID + (mb + 1) * P],
                    rhs=xt_sb[:, kc * N + nb * 512: kc * N + (nb + 1) * 512],
                    start=(kc == 0), stop=(kc == NK - 1),
                )
            nc.vector.tensor_copy(
                qt_sb[:, mb * N + nb * 512: mb * N + (nb + 1) * 512], ps[:])

        for nb in range(NQB):
            k_chain(nb)
        for mb in range(4):  # before V: pair 0 of chunk 0 only needs K+Q
            q_chain(mb, 0)
        for mb in range(NB):
            v_chain(mb)
        # partition-broadcast of the bias row via PE outer product (late so
        # it never blocks the projection stream at the PE FIFO head)
        for jh in range(2):
            bps = ppool.tile([P, 512], F32, tag="proj")
            nc.tensor.matmul(bps[:], lhsT=ones_row[:, 0:P],
                             rhs=bo_row[:, jh * 512:(jh + 1) * 512],
                             start=True, stop=True)
            nc.vector.tensor_copy(bo_bc[:, jh * 512:(jh + 1) * 512], bps[:])

        # ---- attention ---------------------------------------------------
        # hidden^T [512, 2048] bf16, normalized attention outputs.
        # head h lives in q/hid block h%4 at partition half 64*(h//4), which
        # equals its kv head's half in kt (host-reordered weights).
        hid_sb = work.tile([P, 4 * N], BF16, tag="hid")

        avs_tiles = {}  # (h, nb) -> SBUF av tile [128,512] f32
        dpk_tiles = {}  # nb -> [32,128] f32 packed softmax denominators
        filler_q = []  # single-instruction closures woven into the mc loop
        filler_slots = [1]

        def pop_fillers(force_all=False):
            if force_all:
                n = len(filler_q)
            else:
                slots = max(filler_slots[0], 1)
                n = min(-(-len(filler_q) // slots), 3)
                filler_slots[0] -= 1
            for _ in range(n):
                filler_q.pop(0)()

        def attn_pair(p, nb, weave=False):
            """Heads hA=p (kv0, partitions 0:64) and hB=p+4 (kv1, 64:128)."""
            avA = avpool.tile([P, 512], F32, tag="av")
            avB = avpool.tile([P, 512], F32, tag="av")
            for mc in range(NB):
                st = stpool.tile([P, 1024], F32, tag="st")
                nc.tensor.matmul(
                    st[:, 0:512],
                    lhsT=kt_sb[0:64, mc * P:(mc + 1) * P],
                    rhs=qt_sb[0:64, p * N + nb * 512: p * N + (nb + 1) * 512],
                    start=True, stop=True,
                )
                nc.tensor.matmul(
                    st[:, 512:1024],
                    lhsT=kt_sb[64:128, mc * P:(mc + 1) * P],
                    rhs=qt_sb[64:128, p * N + nb * 512: p * N + (nb + 1) * 512],
                    start=True, stop=True,
                )
                pt = ptpool.tile([P, 1024], BF16, tag="pt")
                nc.scalar.activation(pt[:], st[:],
                                     mybir.ActivationFunctionType.Exp,
                                     scale=SCALE)
                nc.tensor.matmul(
                    avA[:],
                    lhsT=v_sb[:, mc * VW: mc * VW + P],
                    rhs=pt[:, 0:512],
                    start=(mc == 0), stop=(mc == NB - 1),
                )
                nc.tensor.matmul(
                    avB[:],
                    lhsT=v_sb[:, mc * VW + P: mc * VW + 2 * P],
                    rhs=pt[:, 512:1024],
                    start=(mc == 0), stop=(mc == NB - 1),
                )
                if weave and filler_q:
                    pop_fillers()
            sA = avsb.tile([P, 512], F32, tag="avs")
            sB = avsb.tile([P, 512], F32, tag="avs")
            nc.vector.tensor_copy(sA[:], avA[:])
            nc.vector.tensor_copy(sB[:], avB[:])
            avs_tiles[(p, nb)] = sA
            avs_tiles[(p + 4, nb)] = sB
            # densify this pair's softmax denominators (row 64 carries them,
            # replicated x64): head h -> dpk rows [4h, 4h+4)
            dpk = dpk_tiles[nb]
            for h in (p, p + 4):
                s = avs_tiles[(h, nb)]
                nc.sync.dma_start(out=dpk[4 * h:4 * h + 4, 0:128],
                                  in_=s[64:65, 0:512])

        def normalize_chunk(nb):
            """One dense [32,128] reciprocal for the whole chunk, then DMA
            each head's row back replicated across 64 partitions and scale
            the attention outputs into hid."""
            dpk = dpk_tiles.pop(nb)
            rcp = rpool.tile([32, 128], F32, tag="rcp")
            nc.vector.reciprocal(rcp[:], dpk[:])
            nc.sync.dma_start(out=rd[nb][0:32, :], in_=rcp[:])
            for h in range(8):
                s = avs_tiles.pop((h, nb))
                rt = rpool.tile([64, 512], F32, tag="rt")
                nc.sync.dma_start(
                    out=rt[:],
                    in_=rd[nb][4 * h:4 * h + 4, :].flatten()[None]
                    .broadcast_to([64, 512]))
                hp = 64 * (h // 4)
                hb = h % 4
                nc.vector.tensor_tensor(
                    out=hid_sb[hp:hp + 64,
                               hb * N + nb * 512: hb * N + (nb + 1) * 512],
                    in0=s[0:64, :], in1=rt[:],
                    op=mybir.AluOpType.mult,
                )

        def oproj_mm(nb, tb, jh, ic, state, psum_pool=None):
            """One matmul of the partial o-proj chain for tokens
            [nb*512 + tb*128, +128), out dims [jh*512, +512); the last one
            also casts the psum to bf16 and ships it to the po buffer."""
            pool = psum_pool if psum_pool is not None else ppool
            if ic == 0:
                state['ps'] = pool.tile(
                    [P, 512], F32,
                    tag=("proj" if pool is ppool else "av"), name="ops")
            ps = state['ps']
            nc.tensor.matmul(
                ps[:],
                lhsT=hid_sb[:, ic * N + nb * 512 + tb * P:
                            ic * N + nb * 512 + (tb + 1) * P],
                rhs=wot_sb[:, ic * D + jh * 512: ic * D + (jh + 1) * 512],
                start=(ic == 0), stop=(ic == 3),
            )
            if ic == 3:
                pob = popool.tile([P, 512], BF16, tag="po")
                nc.vector.tensor_copy(pob[:], ps[:])
                # chunk 3's po rows are interleaved [tb0; tb2; tb1; tb3] so
                # its ReduceScatter can be issued in two overlapping halves
                # while still scattering contiguous token quarters per rank
                rb_ = {0: 0, 2: 1, 1: 2, 3: 3}[tb] if nb == 3 else tb
                nc.sync.dma_start(
                    out=po[nb][rb_ * P:(rb_ + 1) * P, jh * 512:(jh + 1) * 512],
                    in_=pob[:])

        def push_oproj_fillers(nb):
            for tb in range(4):
                for jh in range(2):
                    state = {}
                    for ic in range(4):
                        filler_q.append(
                            lambda nb=nb, tb=tb, jh=jh, ic=ic, state=state:
                            oproj_mm(nb, tb, jh, ic, state))

        def push_q_fillers(nb):
            for mb in range(4):
                state = {}

                def q_mm(kc, mb=mb, nb=nb, state=state):
                    if kc == 0:
                        state['ps'] = ppool.tile([P, 512], F32, tag="proj",
                                                 name="qps")
                    ps = state['ps']
                    nc.tensor.matmul(
                        ps[:],
                        lhsT=wqt_sb[:, kc * HID + mb * P:
                                    kc * HID + (mb + 1) * P],
                        rhs=xt_sb[:, kc * N + nb * 512:
                                  kc * N + (nb + 1) * 512],
                        start=(kc == 0), stop=(kc == NK - 1),
                    )
                    if kc == NK - 1:
                        nc.vector.tensor_copy(
                            qt_sb[:, mb * N + nb * 512:
                                  mb * N + (nb + 1) * 512], ps[:])
                for kc in range(NK):
                    filler_q.append(lambda kc=kc, f=q_mm: f(kc))

        def rs_issue(nb, half=None):
            if half is None:
                r0, r1 = 0, 512
            else:
                r0, r1 = half * OWN, half * OWN + OWN
            nc.gpsimd.collective_compute(
                "ReduceScatter", mybir.AluOpType.add,
                replica_groups=RG,
                ins=[po[nb][r0:r1, :].opt()],
                outs=[rs[nb][r0 // 2:r1 // 2, :].opt()],
            )

        def rs_finish(nb):
            """Read back our 256-token quarter, add bias, write out_p."""
            for tb in range(2):
                rsb = finp.tile([P, D], BF16, tag="rsb")
                nc.sync.dma_start(out=rsb[:], in_=rs[nb][tb * P:(tb + 1) * P, :])
                ot = finp.tile([P, D], F32, tag="ot")
                nc.vector.tensor_tensor(out=ot[:], in0=rsb[:], in1=bo_bc[:],
                                        op=mybir.AluOpType.add)
                r0 = nb * 512 + tb * P
                nc.sync.dma_start(out=out_p[r0:r0 + P, :], in_=ot[:])

        for nb in range(NQB):
            dpk_tiles[nb] = avsb.tile([32, 128], F32, tag="dpk", bufs=2,
                                      name="dpk")
            attn_pair(0, nb)
            if nb > 0:
                normalize_chunk(nb - 1)
            if nb == 0:  # remaining Q projections become fillers
                for nbq in range(1, NQB):
                    push_q_fillers(nbq)
                filler_slots[0] = 3 * NB
                first_weave = 1
            else:
                push_oproj_fillers(nb - 1)
                # delay popping one pair so normalize_chunk(nb-1) finishes
                filler_slots[0] = 2 * NB
                first_weave = 2
            for p in range(1, 4):
                attn_pair(p, nb, weave=(p >= first_weave))
                if p == 1 and nb > 1:
                    # late enough that the pair ReduceScatter has landed, so
                    # these adds never block the DVE FIFO
                    rs_finish(nb - 2)
            pop_fillers(force_all=True)
            if nb > 0:
                rs_issue(nb - 1)
        # tail: finish chunk 3
        normalize_chunk(3)
        rs_finish(2)
        for half, tbs in enumerate(((0, 2), (1, 3))):
            for tb in tbs:
                for jh in range(2):
                    # attention is done; alternate PSUM pools to pipeline
                    state = {}
                    for ic in range(4):
                        oproj_mm(3, tb, jh, ic, state,
                                 psum_pool=(ppool if jh == 0 else avpool))
            rs_issue(3, half=half)
        rs_finish(3)

    _split_excess_waits(nc)
    return nc


def make_in_maps(x, wq, wk, wv, wo, bo):
    bf = ml_dtypes.bfloat16
    # local head h -> device slot (block h%4, half h//4): permuted head order
    hperm = [0, 4, 1, 5, 2, 6, 3, 7]
    dperm = np.concatenate([np.arange(64 * h, 64 * h + 64) for h in hperm])
    in_maps = []
    for c in range(NCORES):
        b, h2 = c // 2, c % 2
        wq_c = wq[HID * h2:HID * (h2 + 1)][dperm]  # [512, 1024] permuted rows
        wot_c = wo.T[HID * h2:HID * (h2 + 1)][dperm]  # [512, 1024] same perm
        in_maps.append({
            "xt": np.ascontiguousarray(x[b].T).astype(bf),
            "wqt": np.ascontiguousarray(wq_c.T).astype(bf),
            "wkt": np.ascontiguousarray(wk[P * h2:P * (h2 + 1)].T).astype(bf),
            "wvt": np.ascontiguousarray(wv[P * h2:P * (h2 + 1)].T).astype(bf),
            "wot": np.ascontiguousarray(wot_c).astype(bf),
            "bo_in": bo.astype(np.float32).reshape(1, D),
        })
    return in_maps


_CACHED_NC = None


def kernel(x, wq, wk, wv, wo, bo, _trace=False, _trace_kwargs=None):
    global _CACHED_NC
    from concourse.bass_utils import run_bass_kernel_spmd

    if _CACHED_NC is None:
        _CACHED_NC = build_nc()
    nc = _CACHED_NC

    in_maps = make_in_maps(
        np.asarray(x, np.float32), np.asarray(wq, np.float32),
        np.asarray(wk, np.float32), np.asarray(wv, np.float32),
        np.asarray(wo, np.float32), np.asarray(bo, np.float32))

    res = run_bass_kernel_spmd(
        nc, in_maps, core_ids=list(range(NCORES)),
        trace=_trace, **(_trace_kwargs or {}))

    out = np.empty((B, N, D), np.float32)
    for b in range(B):
        for h2 in range(2):
            r = res.results[2 * b + h2]["out_p"]
            for nb in range(NQB):
                out[b, nb * 512 + h2 * OWN: nb * 512 + (h2 + 1) * OWN] = \
                    r[nb * 512: nb * 512 + OWN]
    if _trace:
        kernel._last_results = res
    return out


# revision 42
# speedup vs baseline: 1.9019x; 1.0018x over previous
"""GQA attention kernel for 8 TRN2 NeuronCores (v2).

Problem: x[4,2048,1024], 16 Q heads / 4 KV heads, head_dim 64 (torch-Linear
style projections, softmax(QK^T/8)V, output projection + bias).

Sharding: core c handles (batch b = c//2, half h2 = c%2) where a half is
2 KV heads = 8 Q heads = 512 hidden dims. Per 512-token chunk, each core
computes the partial output projection over its 512 hidden dims (bf16,
pre-bias); the pair ReduceScatters it so each member ends up with the final
sum for a disjoint 256-token quarter, adds the bias, and writes those rows
to out_p. The program is identical on all cores (which quarter a core gets
falls out of its replica-group rank); the host stitches by core parity.

Attention inner loop (per query chunk nb, per head-pair (kv0 head, kv1
head)): the two K=64 QK matmuls sit at SBUF partition bases 0/64, so they
auto-derive tile_position (0,0)/(64,0) and run concurrently in disjoint PE
row groups; their [128,512] score tiles land in adjacent PSUM banks and are
exp'd by ONE FD=1024 ACTIVATE (ACT is the bottleneck engine; its ~300-cycle
fixed overhead is amortized). V is augmented with ones columns so the AV
matmul also produces softmax denominators broadcast across 64 partitions
for free. av tiles are copied to SBUF immediately (frees PSUM early) and
the expensive DVE reciprocals (8 cyc/elem) are deferred and woven between
later pairs so they never stall the QK->exp->AV stream.
"""

import sys
import numpy as np
from contextlib import ExitStack

sys.path.insert(0, "/opt/trn_rl_repo")

import ml_dtypes

from concourse import bass, tile, mybir


# ---------------------------------------------------------------------------
# This walrus build encodes at most 1-2 sync waits per instruction; the stock
# TileContext tail drain packs one wait per live proc onto a single Drain and
# fails codegen ("Too many sync wait commands"). Spread the waits over SP nop
# carriers instead.
def _patched_drain_and_barrier(self, tick_clock, wait_clock):
    from concourse.vector_clock import ScopedClock, VectorClock

    nc = self.nc
    gc = tick_clock.global_clock
    n = len(gc)
    for proc in range(n):
        t = gc[proc]
        if t <= 0:
            continue
        carrier = nc.sync.nop(nofuse=True)
        req = VectorClock([t if i == proc else 0 for i in range(n)])
        wait_clock.add_sem_waits(carrier.ins, ScopedClock({None: req}))
    nc.sync.drain()
    nc.all_engine_barrier()
    assert self.sems is not None
    popped = nc._tile_sem_poison_stack.pop()
    assert popped is self._sem_poison
    nc.clear_and_free_semaphores(list(self.sems.allocated().values()))
    nc.all_engine_barrier()


tile.TileContext._drain_and_barrier = _patched_drain_and_barrier


def _split_excess_waits(nc, max_waits=1):
    """Hoist all but one sync wait per instruction onto dedicated
    EventSemaphore carriers placed immediately before it on the same engine
    (same blocking semantics, one wait per encoded instruction)."""
    n_new = 0
    for bb in nc.main_func.blocks:
        il = list(bb.instructions)
        out = []
        changed = False
        for ins in il:
            si = ins.sync_info
            if si is not None:
                w = list(si.on_wait)
                if len(w) > max_waits:
                    for extra in w[max_waits:]:
                        ev = mybir.InstEventSemaphore(
                            name=f"{ins.name}-wsp{n_new}", engine=ins.engine)
                        n_new += 1
                        ev.sync_info = type(si)(on_wait=[extra], on_update=[])
                        nc.register_instruction(ev, overwrite=True)
                        out.append(ev)
                    si.on_wait = w[:max_waits]
                    changed = True
            out.append(ins)
        if changed:
            bb.instructions = out
# ---------------------------------------------------------------------------

B, N, D = 4, 2048, 1024
DH = 64  # head dim
HID = 512  # hidden dims per core (8 q heads)
NCORES = 8
P = 128
SCALE = DH ** -0.5
BF16 = mybir.dt.bfloat16
F32 = mybir.dt.float32

NB = N // P  # 16 key blocks of 128
NK = D // P  # 8 contraction chunks of 128
NQB = 4  # n query chunks of 512
VW = 256  # v chunk width: [64 v_kv0 | 64 ones | 64 v_kv1 | 64 ones]
OWN = 256  # tokens of each 512-chunk this core ends up with after RS

RG = [[0, 1], [2, 3], [4, 5], [6, 7]]


def build_nc():
    nc = bass.Bass(target_bir_lowering=False, debug=False, num_devices=NCORES)

    xt = nc.declare_dram_parameter("xt", [D, N], BF16, isOutput=False)
    wqt = nc.declare_dram_parameter("wqt", [D, HID], BF16, isOutput=False)
    wkt = nc.declare_dram_parameter("wkt", [D, P], BF16, isOutput=False)
    wvt = nc.declare_dram_parameter("wvt", [D, P], BF16, isOutput=False)
    wot = nc.declare_dram_parameter("wot", [HID, D], BF16, isOutput=False)
    ident = nc.declare_dram_parameter("ident", [P, P], BF16, isOutput=False)
    bo_in = nc.declare_dram_parameter("bo_in", [1, D], F32, isOutput=False)
    out_p = nc.declare_dram_parameter("out_p", [N, D], F32, isOutput=True)

    # per-chunk partial o-proj (bf16, pre-bias) and its pair ReduceScatter
    po = [nc.dram_tensor(f"po{k}", [512, D], BF16) for k in range(NQB)]
    rs = [nc.dram_tensor(f"rs{k}", [OWN, D], BF16) for k in range(NQB)]
    # bounce buffer for the packed softmax reciprocals (DRAM so the
    # partition-replicating read-back can use a stride-0 outer dim)
    rd = [nc.dram_tensor(f"rd{k}", [32, 128], F32) for k in range(NQB)]

    with tile.TileContext(nc) as tc, ExitStack() as ctx:
        const = ctx.enter_context(tc.tile_pool(name="const", bufs=1))
        work = ctx.enter_context(tc.tile_pool(name="work", bufs=1))
        # PSUM: st 2x[128,1024] (4 banks) + av 3 + proj 1 = 8 banks
        stpool = ctx.enter_context(tc.tile_pool(name="stp", bufs=2, space="PSUM"))
        avpool = ctx.enter_context(tc.tile_pool(name="avp", bufs=3, space="PSUM"))
        ppool = ctx.enter_context(tc.tile_pool(name="ppool", bufs=1, space="PSUM"))
        ptpool = ctx.enter_context(tc.tile_pool(name="ptp", bufs=3))
        avsb = ctx.enter_context(tc.tile_pool(name="avsb", bufs=12))
        rpool = ctx.enter_context(tc.tile_pool(name="rp", bufs=3))
        popool = ctx.enter_context(tc.tile_pool(name="pop", bufs=3))
        finp = ctx.enter_context(tc.tile_pool(name="finp", bufs=2))

        # ---- load inputs -------------------------------------------------
        xt_sb = const.tile([P, NK * N], BF16)
        wkt_sb = const.tile([P, NK * P], BF16)
        bo_row = const.tile([1, D], F32)
        nc.sync.dma_start(out=bo_row[:], in_=bo_in[0:1, :])
        for kc in range(NK):
            nc.sync.dma_start(out=wkt_sb[:, kc * P:(kc + 1) * P],
                              in_=wkt[kc * P:(kc + 1) * P, :])
            nc.sync.dma_start(out=xt_sb[:, kc * N:(kc + 1) * N],
                              in_=xt[kc * P:(kc + 1) * P, :])
        wqt_sb = const.tile([P, NK * HID], BF16)
        wvt_sb = const.tile([P, NK * P], BF16)
        for kc in range(NK):
            nc.sync.dma_start(out=wqt_sb[:, kc * HID:(kc + 1) * HID],
                              in_=wqt[kc * P:(kc + 1) * P, :])
            nc.sync.dma_start(out=wvt_sb[:, kc * P:(kc + 1) * P],
                              in_=wvt[kc * P:(kc + 1) * P, :])
        ident_sb = const.tile([P, P], BF16)
        nc.sync.dma_start(out=ident_sb[:], in_=ident[0:P, :])
        wot_sb = const.tile([P, 4 * D], BF16)
        for ic in range(4):
            nc.sync.dma_start(out=wot_sb[:, ic * D:(ic + 1) * D],
                              in_=wot[ic * P:(ic + 1) * P, :])
        ones_row = const.tile([1, P], F32)
        nc.vector.memset(ones_row[:], 1.0)
        bo_bc = const.tile([P, D], F32)

        # ---- projections -------------------------------------------------
        # q^T [512, 2048] as 4 partition-blocks; k^T [128, 2048]; v natural.
        qt_sb = work.tile([P, 4 * N], BF16, tag="qt")
        kt_sb = work.tile([P, N], BF16, tag="kt")
        v_sb = work.tile([P, NB * VW], BF16, tag="v")
        nc.vector.memset(v_sb[:], 1.0)  # ones columns survive the copies

        def k_chain(nb):
            ps = avpool.tile([P, 512], F32, tag="av")
            for kc in range(NK):
                nc.tensor.matmul(
                    ps[:],
                    lhsT=wkt_sb[:, kc * P:(kc + 1) * P],
                    rhs=xt_sb[:, kc * N + nb * 512: kc * N + (nb + 1) * 512],
                    start=(kc == 0), stop=(kc == NK - 1),
                )
            nc.vector.tensor_copy(kt_sb[:, nb * 512:(nb + 1) * 512], ps[:])

        # v is projected TRANSPOSED like k (4 wide chains instead of 16
        # LDWEIGHTS-heavy natural-layout chains), then flipped back into
        # v_sb chunk-by-chunk with cheap PE transposes woven into pair 0.
        vt_sb = work.tile([P, N], BF16, tag="vt")

        def vt_chain(nb4):
            ps = avpool.tile([P, 512], F32, tag="av")
            for kc in range(NK):
                nc.tensor.matmul(
                    ps[:],
                    lhsT=wvt_sb[:, kc * P:(kc + 1) * P],
                    rhs=xt_sb[:, kc * N + nb4 * 512: kc * N + (nb4 + 1) * 512],
                    start=(kc == 0), stop=(kc == NK - 1),
                )
            nc.vector.tensor_copy(vt_sb[:, nb4 * 512:(nb4 + 1) * 512], ps[:])

        def v_transpose(mc, state):
            if 't' not in state:
                t = ppool.tile([P, 512], F32, tag="proj", name="vtp")
                state['t'] = t[:, 0:64].bitcast(BF16)  # [128, 128] bf16 view
                nc.tensor.transpose(state['t'],
                                    vt_sb[:, mc * P:(mc + 1) * P], ident_sb[:])
            else:
                ps = state['t']
                nc.vector.tensor_copy(v_sb[:, mc * VW: mc * VW + 64],
                                      ps[:, 0:64])
                nc.vector.tensor_copy(v_sb[:, mc * VW + 128: mc * VW + 192],
                                      ps[:, 64:128])

        def q_chain(mb, nb):
            ps = ppool.tile([P, 512], F32, tag="proj")
            for kc in range(NK):
                nc.tensor.matmul(
                    ps[:],
                    lhsT=wqt_sb[:, kc * HID + mb * P: kc * HID + (mb + 1) * P],
                    rhs=xt_sb[:, kc * N + nb * 512: kc * N + (nb + 1) * 512],
                    start=(kc == 0), stop=(kc == NK - 1),
                )
            nc.vector.tensor_copy(
                qt_sb[:, mb * N + nb * 512: mb * N + (nb + 1) * 512], ps[:])

        for nb in range(NQB):
            k_chain(nb)
        for mb in range(4):  # before V: pair 0 of chunk 0 only needs K+Q
            q_chain(mb, 0)
        for nb4 in range(4):
            vt_chain(nb4)
        for mc in range(4):  # first few v chunks inline; rest are fillers
            st_ = {}
            v_transpose(mc, st_)
            v_transpose(mc, st_)
        # partition-broadcast of the bias row via PE outer product (late so
        # it never blocks the projection stream at the PE FIFO head)
        for jh in range(2):
            bps = ppool.tile([P, 512], F32, tag="proj")
            nc.tensor.matmul(bps[:], lhsT=ones_row[:, 0:P],
                             rhs=bo_row[:, jh * 512:(jh + 1) * 512],
                             start=True, stop=True)
            nc.vector.tensor_copy(bo_bc[:, jh * 512:(jh + 1) * 512], bps[:])

        # ---- attention ---------------------------------------------------
        # hidden^T [512, 2048] bf16, normalized attention outputs.
        # head h lives in q/hid block h%4 at partition half 64*(h//4), which
        # equals its kv head's half in kt (host-reordered weights).
        hid_sb = work.tile([P, 4 * N], BF16, tag="hid")

        avs_tiles = {}  # (h, nb) -> SBUF av tile [128,512] f32
        dpk_tiles = {}  # nb -> [32,128] f32 packed softmax denominators
        filler_q = []  # single-instruction closures woven into the mc loop
        filler_slots = [1]

        def pop_fillers(force_all=False):
            if force_all:
                n = len(filler_q)
            else:
                slots = max(filler_slots[0], 1)
                n = min(-(-len(filler_q) // slots), 3)
                filler_slots[0] -= 1
            for _ in range(n):
                filler_q.pop(0)()

        def attn_pair(p, nb, weave=False):
            """Heads hA=p (kv0, partitions 0:64) and hB=p+4 (kv1, 64:128)."""
            avA = avpool.tile([P, 512], F32, tag="av")
            avB = avpool.tile([P, 512], F32, tag="av")
            for mc in range(NB):
                st = stpool.tile([P, 1024], F32, tag="st")
                nc.tensor.matmul(
                    st[:, 0:512],
                    lhsT=kt_sb[0:64, mc * P:(mc + 1) * P],
                    rhs=qt_sb[0:64, p * N + nb * 512: p * N + (nb + 1) * 512],
                    start=True, stop=True,
                )
                nc.tensor.matmul(
                    st[:, 512:1024],
                    lhsT=kt_sb[64:128, mc * P:(mc + 1) * P],
                    rhs=qt_sb[64:128, p * N + nb * 512: p * N + (nb + 1) * 512],
                    start=True, stop=True,
                )
                pt = ptpool.tile([P, 1024], BF16, tag="pt")
                nc.scalar.activation(pt[:], st[:],
                                     mybir.ActivationFunctionType.Exp,
                                     scale=SCALE)
                nc.tensor.matmul(
                    avA[:],
                    lhsT=v_sb[:, mc * VW: mc * VW + P],
                    rhs=pt[:, 0:512],
                    start=(mc == 0), stop=(mc == NB - 1),
                )
                nc.tensor.matmul(
                    avB[:],
                    lhsT=v_sb[:, mc * VW + P: mc * VW + 2 * P],
                    rhs=pt[:, 512:1024],
                    start=(mc == 0), stop=(mc == NB - 1),
                )
                if weave and filler_q:
                    pop_fillers()
            sA = avsb.tile([P, 512], F32, tag="avs")
            sB = avsb.tile([P, 512], F32, tag="avs")
            nc.vector.tensor_copy(sA[:], avA[:])
            nc.vector.tensor_copy(sB[:], avB[:])
            avs_tiles[(p, nb)] = sA
            avs_tiles[(p + 4, nb)] = sB
            # densify this pair's softmax denominators (row 64 carries them,
            # replicated x64): head h -> dpk rows [4h, 4h+4)
            dpk = dpk_tiles[nb]
            for h in (p, p + 4):
                s = avs_tiles[(h, nb)]
                nc.sync.dma_start(out=dpk[4 * h:4 * h + 4, 0:128],
                                  in_=s[64:65, 0:512])

        rt_tiles = {}

        def normalize_recip(nb):
            """One dense [32,128] reciprocal for the whole chunk, bounced
            through DRAM and DMA'd back replicated across 64 partitions.
            Issued right after the chunk's last pair so the ~10us DMA queue
            latency is absorbed under the next chunk's first pair."""
            dpk = dpk_tiles.pop(nb)
            rcp = rpool.tile([32, 128], F32, tag="rcp")
            nc.vector.reciprocal(rcp[:], dpk[:])
            nc.sync.dma_start(out=rd[nb][0:32, :], in_=rcp[:])
            rts = []
            for h in range(8):
                rt = rpool.tile([64, 512], F32, tag="rt", bufs=9)
                nc.sync.dma_start(
                    out=rt[:],
                    in_=rd[nb][4 * h:4 * h + 4, :].flatten()[None]
                    .broadcast_to([64, 512]))
                rts.append(rt)
            rt_tiles[nb] = rts

        def normalize_mults(nb):
            rts = rt_tiles.pop(nb)
            for h in range(8):
                s = avs_tiles.pop((h, nb))
                hp = 64 * (h // 4)
                hb = h % 4
                nc.vector.tensor_tensor(
                    out=hid_sb[hp:hp + 64,
                               hb * N + nb * 512: hb * N + (nb + 1) * 512],
                    in0=s[0:64, :], in1=rts[h][:],
                    op=mybir.AluOpType.mult,
                )

        def oproj_mm(nb, tb, jh, ic, state, psum_pool=None):
            """One matmul of the partial o-proj chain for tokens
            [nb*512 + tb*128, +128), out dims [jh*512, +512); the last one
            also casts the psum to bf16 and ships it to the po buffer."""
            pool = psum_pool if psum_pool is not None else ppool
            if ic == 0:
                state['ps'] = pool.tile(
                    [P, 512], F32,
                    tag=("proj" if pool is ppool else "av"), name="ops")
            ps = state['ps']
            nc.tensor.matmul(
                ps[:],
                lhsT=hid_sb[:, ic * N + nb * 512 + tb * P:
                            ic * N + nb * 512 + (tb + 1) * P],
                rhs=wot_sb[:, ic * D + jh * 512: ic * D + (jh + 1) * 512],
                start=(ic == 0), stop=(ic == 3),
            )
            if ic == 3:
                pob = popool.tile([P, 512], BF16, tag="po")
                nc.vector.tensor_copy(pob[:], ps[:])
                nc.sync.dma_start(
                    out=po[nb][tb * P:(tb + 1) * P, jh * 512:(jh + 1) * 512],
                    in_=pob[:])

        def push_oproj_fillers(nb):
            for tb in range(4):
                for jh in range(2):
                    state = {}
                    for ic in range(4):
                        filler_q.append(
                            lambda nb=nb, tb=tb, jh=jh, ic=ic, state=state:
                            oproj_mm(nb, tb, jh, ic, state))

        def push_q_fillers(nb):
            for mb in range(4):
                state = {}

                def q_mm(kc, mb=mb, nb=nb, state=state):
                    if kc == 0:
                        state['ps'] = ppool.tile([P, 512], F32, tag="proj",
                                                 name="qps")
                    ps = state['ps']
                    nc.tensor.matmul(
                        ps[:],
                        lhsT=wqt_sb[:, kc * HID + mb * P:
                                    kc * HID + (mb + 1) * P],
                        rhs=xt_sb[:, kc * N + nb * 512:
                                  kc * N + (nb + 1) * 512],
                        start=(kc == 0), stop=(kc == NK - 1),
                    )
                    if kc == NK - 1:
                        nc.vector.tensor_copy(
                            qt_sb[:, mb * N + nb * 512:
                                  mb * N + (nb + 1) * 512], ps[:])
                for kc in range(NK):
                    filler_q.append(lambda kc=kc, f=q_mm: f(kc))

        def rs_issue(nb, half=None):
            if half is None:
                r0, r1 = 0, 512
            else:
                r0, r1 = half * OWN, half * OWN + OWN
            nc.gpsimd.collective_compute(
                "ReduceScatter", mybir.AluOpType.add,
                replica_groups=RG,
                ins=[po[nb][r0:r1, :].opt()],
                outs=[rs[nb][r0 // 2:r1 // 2, :].opt()],
            )

        def rs_finish(nb):
            """Read back our 256-token quarter, add bias, write out_p."""
            for tb in range(2):
                rsb = finp.tile([P, D], BF16, tag="rsb")
                nc.sync.dma_start(out=rsb[:], in_=rs[nb][tb * P:(tb + 1) * P, :])
                ot = finp.tile([P, D], F32, tag="ot")
                nc.vector.tensor_tensor(out=ot[:], in0=rsb[:], in1=bo_bc[:],
                                        op=mybir.AluOpType.add)
                r0 = nb * 512 + tb * P
                nc.sync.dma_start(out=out_p[r0:r0 + P, :], in_=ot[:])

        for nb in range(NQB):
            dpk_tiles[nb] = avsb.tile([32, 128], F32, tag="dpk", bufs=2,
                                      name="dpk")
            if nb == 0:
                # remaining v transposes and Q projections become fillers
                for mc in range(4, NB):
                    st_ = {}
                    for _ in range(2):
                        filler_q.append(
                            lambda mc=mc, st_=st_: v_transpose(mc, st_))
                for nbq in range(1, NQB):
                    push_q_fillers(nbq)
                filler_slots[0] = 4 * NB
                first_weave = 0
            attn_pair(0, nb, weave=(nb == 0))
            if nb > 0:
                normalize_mults(nb - 1)
                push_oproj_fillers(nb - 1)
                # delay popping one pair so the hid mults land first
                filler_slots[0] = 2 * NB
                first_weave = 2
            for p in range(1, 4):
                attn_pair(p, nb, weave=(nb == 0 or p >= first_weave))
                if p == 1 and nb > 1:
                    # late enough that the pair ReduceScatter has landed, so
                    # these adds never block the DVE FIFO
                    rs_finish(nb - 2)
            normalize_recip(nb)
            pop_fillers(force_all=True)
            if nb > 0:
                rs_issue(nb - 1)
        # tail: finish chunk 3
        rs_finish(2)
        normalize_mults(3)
        for tb in range(4):
            for jh in range(2):
                # attention is done; alternate PSUM pools to pipeline
                state = {}
                for ic in range(4):
                    oproj_mm(3, tb, jh, ic, state,
                             psum_pool=(ppool if jh == 0 else avpool))
        rs_issue(3)
        rs_finish(3)

    _split_excess_waits(nc)
    return nc


def make_in_maps(x, wq, wk, wv, wo, bo):
    bf = ml_dtypes.bfloat16
    # local head h -> device slot (block h%4, half h//4): permuted head order
    hperm = [0, 4, 1, 5, 2, 6, 3, 7]
    dperm = np.concatenate([np.arange(64 * h, 64 * h + 64) for h in hperm])
    in_maps = []
    for c in range(NCORES):
        b, h2 = c // 2, c % 2
        wq_c = wq[HID * h2:HID * (h2 + 1)][dperm]  # [512, 1024] permuted rows
        wot_c = wo.T[HID * h2:HID * (h2 + 1)][dperm]  # [512, 1024] same perm
        in_maps.append({
            "xt": np.ascontiguousarray(x[b].T).astype(bf),
            "wqt": np.ascontiguousarray(wq_c.T).astype(bf),
            "wkt": np.ascontiguousarray(wk[P * h2:P * (h2 + 1)].T).astype(bf),
            "wvt": np.ascontiguousarray(wv[P * h2:P * (h2 + 1)].T).astype(bf),
            "wot": np.ascontiguousarray(wot_c).astype(bf),
            "ident": np.eye(P, dtype=bf),
            "bo_in": bo.astype(np.float32).reshape(1, D),
        })
    return in_maps


_CACHED_NC = None


def kernel(x, wq, wk, wv, wo, bo, _trace=False, _trace_kwargs=None):
    global _CACHED_NC
    from concourse.bass_utils import run_bass_kernel_spmd

    if _CACHED_NC is None:
        _CACHED_NC = build_nc()
    nc = _CACHED_NC

    in_maps = make_in_maps(
        np.asarray(x, np.float32), np.asarray(wq, np.float32),
        np.asarray(wk, np.float32), np.asarray(wv, np.float32),
        np.asarray(wo, np.float32), np.asarray(bo, np.float32))

    res = run_bass_kernel_spmd(
        nc, in_maps, core_ids=list(range(NCORES)),
        trace=_trace, **(_trace_kwargs or {}))

    out = np.empty((B, N, D), np.float32)
    for b in range(B):
        for h2 in range(2):
            r = res.results[2 * b + h2]["out_p"]
            for nb in range(NQB):
                out[b, nb * 512 + h2 * OWN: nb * 512 + (h2 + 1) * OWN] = \
                    r[nb * 512: nb * 512 + OWN]
    if _trace:
        kernel._last_results = res
    return out


# revision 45
# speedup vs baseline: 1.9138x; 1.0063x over previous
"""GQA attention kernel for 8 TRN2 NeuronCores (v2).

Problem: x[4,2048,1024], 16 Q heads / 4 KV heads, head_dim 64 (torch-Linear
style projections, softmax(QK^T/8)V, output projection + bias).

Sharding: core c handles (batch b = c//2, half h2 = c%2) where a half is
2 KV heads = 8 Q heads = 512 hidden dims. Per 512-token chunk, each core
computes the partial output projection over its 512 hidden dims (bf16,
pre-bias); the pair ReduceScatters it so each member ends up with the final
sum for a disjoint 256-token quarter, adds the bias, and writes those rows
to out_p. The program is identical on all cores (which quarter a core gets
falls out of its replica-group rank); the host stitches by core parity.

Attention inner loop (per query chunk nb, per head-pair (kv0 head, kv1
head)): the two K=64 QK matmuls sit at SBUF partition bases 0/64, so they
auto-derive tile_position (0,0)/(64,0) and run concurrently in disjoint PE
row groups; their [128,512] score tiles land in adjacent PSUM banks and are
exp'd by ONE FD=1024 ACTIVATE (ACT is the bottleneck engine; its ~300-cycle
fixed overhead is amortized). V is augmented with ones columns so the AV
matmul also produces softmax denominators broadcast across 64 partitions
for free. av tiles are copied to SBUF immediately (frees PSUM early) and
the expensive DVE reciprocals (8 cyc/elem) are deferred and woven between
later pairs so they never stall the QK->exp->AV stream.
"""

import sys
import numpy as np
from contextlib import ExitStack

sys.path.insert(0, "/opt/trn_rl_repo")

import ml_dtypes

from concourse import bass, tile, mybir


# ---------------------------------------------------------------------------
# This walrus build encodes at most 1-2 sync waits per instruction; the stock
# TileContext tail drain packs one wait per live proc onto a single Drain and
# fails codegen ("Too many sync wait commands"). Spread the waits over SP nop
# carriers instead.
def _patched_drain_and_barrier(self, tick_clock, wait_clock):
    from concourse.vector_clock import ScopedClock, VectorClock

    nc = self.nc
    gc = tick_clock.global_clock
    n = len(gc)
    for proc in range(n):
        t = gc[proc]
        if t <= 0:
            continue
        carrier = nc.sync.nop(nofuse=True)
        req = VectorClock([t if i == proc else 0 for i in range(n)])
        wait_clock.add_sem_waits(carrier.ins, ScopedClock({None: req}))
    nc.sync.drain()
    nc.all_engine_barrier()
    assert self.sems is not None
    popped = nc._tile_sem_poison_stack.pop()
    assert popped is self._sem_poison
    nc.clear_and_free_semaphores(list(self.sems.allocated().values()))
    nc.all_engine_barrier()


tile.TileContext._drain_and_barrier = _patched_drain_and_barrier


def _split_excess_waits(nc, max_waits=1):
    """Hoist all but one sync wait per instruction onto dedicated
    EventSemaphore carriers placed immediately before it on the same engine
    (same blocking semantics, one wait per encoded instruction)."""
    n_new = 0
    for bb in nc.main_func.blocks:
        il = list(bb.instructions)
        out = []
        changed = False
        for ins in il:
            si = ins.sync_info
            if si is not None:
                w = list(si.on_wait)
                if len(w) > max_waits:
                    for extra in w[max_waits:]:
                        ev = mybir.InstEventSemaphore(
                            name=f"{ins.name}-wsp{n_new}", engine=ins.engine)
                        n_new += 1
                        ev.sync_info = type(si)(on_wait=[extra], on_update=[])
                        nc.register_instruction(ev, overwrite=True)
                        out.append(ev)
                    si.on_wait = w[:max_waits]
                    changed = True
            out.append(ins)
        if changed:
            bb.instructions = out
# ---------------------------------------------------------------------------

B, N, D = 4, 2048, 1024
DH = 64  # head dim
HID = 512  # hidden dims per core (8 q heads)
NCORES = 8
P = 128
SCALE = DH ** -0.5
BF16 = mybir.dt.bfloat16
F32 = mybir.dt.float32

NB = N // P  # 16 key blocks of 128
NK = D // P  # 8 contraction chunks of 128
NQB = 4  # n query chunks of 512
VW = 256  # v chunk width: [64 v_kv0 | 64 ones | 64 v_kv1 | 64 ones]
OWN = 256  # tokens of each 512-chunk this core ends up with after RS

RG = [[0, 1], [2, 3], [4, 5], [6, 7]]


def build_nc():
    nc = bass.Bass(target_bir_lowering=False, debug=False, num_devices=NCORES)

    xt = nc.declare_dram_parameter("xt", [D, N], BF16, isOutput=False)
    wqt = nc.declare_dram_parameter("wqt", [D, HID], BF16, isOutput=False)
    wkt = nc.declare_dram_parameter("wkt", [D, P], BF16, isOutput=False)
    wvt = nc.declare_dram_parameter("wvt", [D, P], BF16, isOutput=False)
    wot = nc.declare_dram_parameter("wot", [HID, D], BF16, isOutput=False)
    ident = nc.declare_dram_parameter("ident", [P, P], BF16, isOutput=False)
    bo_in = nc.declare_dram_parameter("bo_in", [1, D], F32, isOutput=False)
    out_p = nc.declare_dram_parameter("out_p", [N, D], F32, isOutput=True)

    # per-chunk partial o-proj (bf16, pre-bias) and its pair ReduceScatter
    po = [nc.dram_tensor(f"po{k}", [512, D], BF16) for k in range(NQB)]
    rs = [nc.dram_tensor(f"rs{k}", [OWN, D], BF16) for k in range(NQB)]
    # bounce buffer for the packed softmax reciprocals (DRAM so the
    # partition-replicating read-back can use a stride-0 outer dim)
    rd = [nc.dram_tensor(f"rd{k}", [32, 128], F32) for k in range(NQB)]

    with tile.TileContext(nc) as tc, ExitStack() as ctx:
        const = ctx.enter_context(tc.tile_pool(name="const", bufs=1))
        work = ctx.enter_context(tc.tile_pool(name="work", bufs=1))
        # PSUM: st 2x[128,1024] (4 banks) + av 3 + proj 1 = 8 banks
        stpool = ctx.enter_context(tc.tile_pool(name="stp", bufs=2, space="PSUM"))
        avpool = ctx.enter_context(tc.tile_pool(name="avp", bufs=3, space="PSUM"))
        ppool = ctx.enter_context(tc.tile_pool(name="ppool", bufs=1, space="PSUM"))
        ptpool = ctx.enter_context(tc.tile_pool(name="ptp", bufs=3))
        avsb = ctx.enter_context(tc.tile_pool(name="avsb", bufs=12))
        rpool = ctx.enter_context(tc.tile_pool(name="rp", bufs=3))
        popool = ctx.enter_context(tc.tile_pool(name="pop", bufs=3))
        finp = ctx.enter_context(tc.tile_pool(name="finp", bufs=2))

        # ---- load inputs -------------------------------------------------
        xt_sb = const.tile([P, NK * N], BF16)
        wkt_sb = const.tile([P, NK * P], BF16)
        bo_row = const.tile([1, D], F32)
        nc.sync.dma_start(out=bo_row[:], in_=bo_in[0:1, :])
        for kc in range(NK):
            nc.sync.dma_start(out=wkt_sb[:, kc * P:(kc + 1) * P],
                              in_=wkt[kc * P:(kc + 1) * P, :])
            nc.sync.dma_start(out=xt_sb[:, kc * N:(kc + 1) * N],
                              in_=xt[kc * P:(kc + 1) * P, :])
        wqt_sb = const.tile([P, NK * HID], BF16)
        wvt_sb = const.tile([P, NK * P], BF16)
        for kc in range(NK):
            nc.sync.dma_start(out=wqt_sb[:, kc * HID:(kc + 1) * HID],
                              in_=wqt[kc * P:(kc + 1) * P, :])
            nc.sync.dma_start(out=wvt_sb[:, kc * P:(kc + 1) * P],
                              in_=wvt[kc * P:(kc + 1) * P, :])
        ident_sb = const.tile([P, P], BF16)
        nc.sync.dma_start(out=ident_sb[:], in_=ident[0:P, :])
        wot_sb = const.tile([P, 4 * D], BF16)
        for ic in range(4):
            nc.sync.dma_start(out=wot_sb[:, ic * D:(ic + 1) * D],
                              in_=wot[ic * P:(ic + 1) * P, :])
        ones_row = const.tile([1, P], F32)
        nc.vector.memset(ones_row[:], 1.0)
        bo_bc = const.tile([P, D], F32)

        # ---- projections -------------------------------------------------
        # q^T [512, 2048] as 4 partition-blocks; k^T [128, 2048]; v natural.
        qt_sb = work.tile([P, 4 * N], BF16, tag="qt")
        kt_sb = work.tile([P, N], BF16, tag="kt")
        v_sb = work.tile([P, NB * VW], BF16, tag="v")
        nc.vector.memset(v_sb[:], 1.0)  # ones columns survive the copies

        def k_chain(nb):
            ps = avpool.tile([P, 512], F32, tag="av")
            for kc in range(NK):
                nc.tensor.matmul(
                    ps[:],
                    lhsT=wkt_sb[:, kc * P:(kc + 1) * P],
                    rhs=xt_sb[:, kc * N + nb * 512: kc * N + (nb + 1) * 512],
                    start=(kc == 0), stop=(kc == NK - 1),
                )
            nc.vector.tensor_copy(kt_sb[:, nb * 512:(nb + 1) * 512], ps[:])

        # v is projected TRANSPOSED like k (4 wide chains instead of 16
        # LDWEIGHTS-heavy natural-layout chains), then flipped back into
        # v_sb chunk-by-chunk with cheap PE transposes woven into pair 0.
        vt_sb = work.tile([P, N], BF16, tag="vt")

        def vt_chain(nb4):
            ps = avpool.tile([P, 512], F32, tag="av")
            for kc in range(NK):
                nc.tensor.matmul(
                    ps[:],
                    lhsT=wvt_sb[:, kc * P:(kc + 1) * P],
                    rhs=xt_sb[:, kc * N + nb4 * 512: kc * N + (nb4 + 1) * 512],
                    start=(kc == 0), stop=(kc == NK - 1),
                )
            nc.vector.tensor_copy(vt_sb[:, nb4 * 512:(nb4 + 1) * 512], ps[:])

        def v_transpose(mc, state):
            if 't' not in state:
                t = ppool.tile([P, 512], F32, tag="proj", name="vtp")
                state['t'] = t[:, 0:64].bitcast(BF16)  # [128, 128] bf16 view
                nc.tensor.transpose(state['t'],
                                    vt_sb[:, mc * P:(mc + 1) * P], ident_sb[:])
            else:
                ps = state['t']
                nc.vector.tensor_copy(v_sb[:, mc * VW: mc * VW + 64],
                                      ps[:, 0:64])
                nc.vector.tensor_copy(v_sb[:, mc * VW + 128: mc * VW + 192],
                                      ps[:, 64:128])

        def q_chain(mb, nb):
            ps = ppool.tile([P, 512], F32, tag="proj")
            for kc in range(NK):
                nc.tensor.matmul(
                    ps[:],
                    lhsT=wqt_sb[:, kc * HID + mb * P: kc * HID + (mb + 1) * P],
                    rhs=xt_sb[:, kc * N + nb * 512: kc * N + (nb + 1) * 512],
                    start=(kc == 0), stop=(kc == NK - 1),
                )
            nc.vector.tensor_copy(
                qt_sb[:, mb * N + nb * 512: mb * N + (nb + 1) * 512], ps[:])

        for nb in range(NQB):
            k_chain(nb)
        for mb in range(4):  # before V: pair 0 of chunk 0 only needs K+Q
            q_chain(mb, 0)
        for nb4 in range(4):
            vt_chain(nb4)
        for mc in range(4):  # first few v chunks inline; rest are fillers
            st_ = {}
            v_transpose(mc, st_)
            v_transpose(mc, st_)
        # partition-broadcast of the bias row via PE outer product (late so
        # it never blocks the projection stream at the PE FIFO head)
        for jh in range(2):
            bps = ppool.tile([P, 512], F32, tag="proj")
            nc.tensor.matmul(bps[:], lhsT=ones_row[:, 0:P],
                             rhs=bo_row[:, jh * 512:(jh + 1) * 512],
                             start=True, stop=True)
            nc.vector.tensor_copy(bo_bc[:, jh * 512:(jh + 1) * 512], bps[:])

        # ---- attention ---------------------------------------------------
        # hidden^T [512, 2048] bf16, normalized attention outputs.
        # head h lives in q/hid block h%4 at partition half 64*(h//4), which
        # equals its kv head's half in kt (host-reordered weights).
        hid_sb = work.tile([P, 4 * N], BF16, tag="hid")

        avs_tiles = {}  # (h, nb) -> SBUF av tile [128,512] f32
        dpk_tiles = {}  # nb -> [32,128] f32 packed softmax denominators
        filler_q = []  # single-instruction closures woven into the mc loop
        filler_slots = [1]

        def pop_fillers(force_all=False):
            if force_all:
                n = len(filler_q)
            else:
                slots = max(filler_slots[0], 1)
                n = min(-(-len(filler_q) // slots), 3)
                filler_slots[0] -= 1
            for _ in range(n):
                filler_q.pop(0)()

        def attn_pair(p, nb, weave=False):
            """Heads hA=p (kv0, partitions 0:64) and hB=p+4 (kv1, 64:128)."""
            avA = avpool.tile([P, 512], F32, tag="av")
            avB = avpool.tile([P, 512], F32, tag="av")
            for mc in range(NB):
                st = stpool.tile([P, 1024], F32, tag="st")
                nc.tensor.matmul(
                    st[:, 0:512],
                    lhsT=kt_sb[0:64, mc * P:(mc + 1) * P],
                    rhs=qt_sb[0:64, p * N + nb * 512: p * N + (nb + 1) * 512],
                    start=True, stop=True,
                )
                nc.tensor.matmul(
                    st[:, 512:1024],
                    lhsT=kt_sb[64:128, mc * P:(mc + 1) * P],
                    rhs=qt_sb[64:128, p * N + nb * 512: p * N + (nb + 1) * 512],
                    start=True, stop=True,
                )
                pt = ptpool.tile([P, 1024], BF16, tag="pt")
                nc.scalar.activation(pt[:], st[:],
                                     mybir.ActivationFunctionType.Exp,
                                     scale=SCALE)
                nc.tensor.matmul(
                    avA[:],
                    lhsT=v_sb[:, mc * VW: mc * VW + P],
                    rhs=pt[:, 0:512],
                    start=(mc == 0), stop=(mc == NB - 1),
                )
                nc.tensor.matmul(
                    avB[:],
                    lhsT=v_sb[:, mc * VW + P: mc * VW + 2 * P],
                    rhs=pt[:, 512:1024],
                    start=(mc == 0), stop=(mc == NB - 1),
                )
                if weave and filler_q:
                    pop_fillers()
            sA = avsb.tile([P, 512], F32, tag="avs")
            sB = avsb.tile([P, 512], F32, tag="avs")
            nc.vector.tensor_copy(sA[:], avA[:])
            nc.vector.tensor_copy(sB[:], avB[:])
            avs_tiles[(p, nb)] = sA
            avs_tiles[(p + 4, nb)] = sB
            # densify this pair's softmax denominators (row 64 carries them,
            # replicated x64): head h -> dpk rows [4h, 4h+4)
            dpk = dpk_tiles[nb]
            for h in (p, p + 4):
                s = avs_tiles[(h, nb)]
                nc.sync.dma_start(out=dpk[4 * h:4 * h + 4, 0:128],
                                  in_=s[64:65, 0:512])

        rt_tiles = {}
        ALLH = tuple(range(8))

        def normalize_recip(nb, heads=ALLH):
            """One dense [32,128] reciprocal for the chunk, bounced through
            DRAM and DMA'd back replicated across 64 partitions. Issued
            right after the chunk's last pair so the ~10us DMA queue
            latency is absorbed under the next chunk's first pair."""
            dpk = dpk_tiles[nb]
            hi = 4 * max(heads) + 4
            rcp = rpool.tile([32, 128], F32, tag="rcp")
            nc.vector.reciprocal(rcp[0:hi, :], dpk[0:hi, :])
            nc.sync.dma_start(out=rd[nb][0:hi, :], in_=rcp[0:hi, :])
            for h in heads:
                rt = rpool.tile([64, 512], F32, tag="rt", bufs=9)
                nc.sync.dma_start(
                    out=rt[:],
                    in_=rd[nb][4 * h:4 * h + 4, :].flatten()[None]
                    .broadcast_to([64, 512]))
                rt_tiles[(nb, h)] = rt

        def one_mult(nb, h, rt_ap):
            s = avs_tiles.pop((h, nb))
            hp = 64 * (h // 4)
            hb = h % 4
            nc.vector.tensor_tensor(
                out=hid_sb[hp:hp + 64,
                           hb * N + nb * 512: hb * N + (nb + 1) * 512],
                in0=s[0:64, :], in1=rt_ap,
                op=mybir.AluOpType.mult,
            )

        def normalize_mults(nb, heads=ALLH):
            for h in heads:
                one_mult(nb, h, rt_tiles.pop((nb, h))[:])

        def normalize_direct(nb, h):
            """Single-head normalize with a plain DVE reciprocal: slower on
            DVE but zero DMA hops -- used for the last two heads of the
            final chunk where DMA latency would sit on the kernel tail."""
            s = avs_tiles[(h, nb)]
            rt = rpool.tile([64, 512], F32, tag="rt", bufs=9)
            nc.vector.reciprocal(rt[:], s[64:128, :])
            one_mult(nb, h, rt[:])

        def oproj_mm(nb, tb, jh, ic, state, psum_pool=None):
            """One matmul of the partial o-proj chain for tokens
            [nb*512 + tb*128, +128), out dims [jh*512, +512); the last one
            also casts the psum to bf16 and ships it to the po buffer."""
            pool = psum_pool if psum_pool is not None else ppool
            if ic == 0:
                state['ps'] = pool.tile(
                    [P, 512], F32,
                    tag=("proj" if pool is ppool else "av"), name="ops")
            ps = state['ps']
            nc.tensor.matmul(
                ps[:],
                lhsT=hid_sb[:, ic * N + nb * 512 + tb * P:
                            ic * N + nb * 512 + (tb + 1) * P],
                rhs=wot_sb[:, ic * D + jh * 512: ic * D + (jh + 1) * 512],
                start=(ic == 0), stop=(ic == 3),
            )
            if ic == 3:
                pob = popool.tile([P, 512], BF16, tag="po")
                nc.vector.tensor_copy(pob[:], ps[:])
                nc.sync.dma_start(
                    out=po[nb][tb * P:(tb + 1) * P, jh * 512:(jh + 1) * 512],
                    in_=pob[:])

        def push_oproj_fillers(nb):
            for tb in range(4):
                for jh in range(2):
                    state = {}
                    for ic in range(4):
                        filler_q.append(
                            lambda nb=nb, tb=tb, jh=jh, ic=ic, state=state:
                            oproj_mm(nb, tb, jh, ic, state))

        def push_q_fillers(nb):
            for mb in range(4):
                state = {}

                def q_mm(kc, mb=mb, nb=nb, state=state):
                    if kc == 0:
                        state['ps'] = ppool.tile([P, 512], F32, tag="proj",
                                                 name="qps")
                    ps = state['ps']
                    nc.tensor.matmul(
                        ps[:],
                        lhsT=wqt_sb[:, kc * HID + mb * P:
                                    kc * HID + (mb + 1) * P],
                        rhs=xt_sb[:, kc * N + nb * 512:
                                  kc * N + (nb + 1) * 512],
                        start=(kc == 0), stop=(kc == NK - 1),
                    )
                    if kc == NK - 1:
                        nc.vector.tensor_copy(
                            qt_sb[:, mb * N + nb * 512:
                                  mb * N + (nb + 1) * 512], ps[:])
                for kc in range(NK):
                    filler_q.append(lambda kc=kc, f=q_mm: f(kc))

        def rs_issue(nb, half=None):
            if half is None:
                r0, r1 = 0, 512
            else:
                r0, r1 = half * OWN, half * OWN + OWN
            nc.gpsimd.collective_compute(
                "ReduceScatter", mybir.AluOpType.add,
                replica_groups=RG,
                ins=[po[nb][r0:r1, :].opt()],
                outs=[rs[nb][r0 // 2:r1 // 2, :].opt()],
            )

        def rs_finish(nb):
            """Read back our 256-token quarter, add bias, write out_p."""
            for tb in range(2):
                rsb = finp.tile([P, D], BF16, tag="rsb")
                nc.sync.dma_start(out=rsb[:], in_=rs[nb][tb * P:(tb + 1) * P, :])
                ot = finp.tile([P, D], F32, tag="ot")
                nc.vector.tensor_tensor(out=ot[:], in0=rsb[:], in1=bo_bc[:],
                                        op=mybir.AluOpType.add)
                r0 = nb * 512 + tb * P
                nc.sync.dma_start(out=out_p[r0:r0 + P, :], in_=ot[:])

        for nb in range(NQB):
            dpk_tiles[nb] = avsb.tile([32, 128], F32, tag="dpk", bufs=2,
                                      name="dpk")
            if nb == 0:
                # remaining v transposes and Q projections become fillers
                for mc in range(4, NB):
                    st_ = {}
                    for _ in range(2):
                        filler_q.append(
                            lambda mc=mc, st_=st_: v_transpose(mc, st_))
                for nbq in range(1, NQB):
                    push_q_fillers(nbq)
                filler_slots[0] = 4 * NB
                first_weave = 0
            attn_pair(0, nb, weave=(nb == 0))
            if nb > 0:
                normalize_mults(nb - 1)
                push_oproj_fillers(nb - 1)
                # delay popping one pair so the hid mults land first
                filler_slots[0] = 2 * NB
                first_weave = 2
            if nb > 1:
                # one pair after chunk nb-1's recip DMAs went out: the
                # collective no longer fights them for the DMA rings
                rs_issue(nb - 2)
            for p in range(1, 4):
                attn_pair(p, nb, weave=(nb == 0 or p >= first_weave))
                if p == 2 and nb > 1:
                    # two pairs after rs_issue: the scatter has landed, so
                    # these adds never block the DVE FIFO
                    rs_finish(nb - 2)
                if p == 2 and nb == 3:
                    # chunk 3's dense recips for the six heads already done
                    # (their DMA latency hides under pair 3); heads 3/7 get
                    # direct DVE reciprocals at the tail
                    normalize_recip(3, heads=(0, 1, 2, 4, 5, 6))
            if nb < 3:
                normalize_recip(nb)
            pop_fillers(force_all=True)
        # tail: finish chunk 3
        normalize_mults(3, heads=(0, 1, 2, 4, 5, 6))
        rs_issue(2)
        normalize_direct(3, 3)
        normalize_direct(3, 7)
        for tb in range(4):
            for jh in range(2):
                # attention is done; alternate PSUM pools to pipeline
                state = {}
                for ic in range(4):
                    oproj_mm(3, tb, jh, ic, state,
                             psum_pool=(ppool if jh == 0 else avpool))
        rs_issue(3)
        rs_finish(2)
        rs_finish(3)

    _split_excess_waits(nc)
    return nc


def make_in_maps(x, wq, wk, wv, wo, bo):
    bf = ml_dtypes.bfloat16
    # local head h -> device slot (block h%4, half h//4): permuted head order
    hperm = [0, 4, 1, 5, 2, 6, 3, 7]
    dperm = np.concatenate([np.arange(64 * h, 64 * h + 64) for h in hperm])
    in_maps = []
    for c in range(NCORES):
        b, h2 = c // 2, c % 2
        wq_c = wq[HID * h2:HID * (h2 + 1)][dperm]  # [512, 1024] permuted rows
        wot_c = wo.T[HID * h2:HID * (h2 + 1)][dperm]  # [512, 1024] same perm
        in_maps.append({
            "xt": np.ascontiguousarray(x[b].T).astype(bf),
            "wqt": np.ascontiguousarray(wq_c.T).astype(bf),
            "wkt": np.ascontiguousarray(wk[P * h2:P * (h2 + 1)].T).astype(bf),
            "wvt": np.ascontiguousarray(wv[P * h2:P * (h2 + 1)].T).astype(bf),
            "wot": np.ascontiguousarray(wot_c).astype(bf),
            "ident": np.eye(P, dtype=bf),
            "bo_in": bo.astype(np.float32).reshape(1, D),
        })
    return in_maps


_CACHED_NC = None


def kernel(x, wq, wk, wv, wo, bo, _trace=False, _trace_kwargs=None):
    global _CACHED_NC
    from concourse.bass_utils import run_bass_kernel_spmd

    if _CACHED_NC is None:
        _CACHED_NC = build_nc()
    nc = _CACHED_NC

    in_maps = make_in_maps(
        np.asarray(x, np.float32), np.asarray(wq, np.float32),
        np.asarray(wk, np.float32), np.asarray(wv, np.float32),
        np.asarray(wo, np.float32), np.asarray(bo, np.float32))

    res = run_bass_kernel_spmd(
        nc, in_maps, core_ids=list(range(NCORES)),
        trace=_trace, **(_trace_kwargs or {}))

    out = np.empty((B, N, D), np.float32)
    for b in range(B):
        for h2 in range(2):
            r = res.results[2 * b + h2]["out_p"]
            for nb in range(NQB):
                out[b, nb * 512 + h2 * OWN: nb * 512 + (h2 + 1) * OWN] = \
                    r[nb * 512: nb * 512 + OWN]
    if _trace:
        kernel._last_results = res
    return out
